# revision 21
# baseline (speedup 1.0000x reference)
"""Tensor-parallel DeepSpeed-style self-attention block on 8 TRN2 NeuronCores.

v4 strategy (fold LN into GEMM epilogues; kill startup + queue serialization):
  - Host pre-transposes the input to xT (bf16) and also passes a natural
    bf16 copy for LayerNorm statistics.  The QKV GEMM runs directly on the
    RAW xT -- no z pass, no on-device zT transposes -- so the first matmul
    fires as soon as the first weight/xT chunks land (~10us vs ~67us).
  - LN is applied algebraically in the PSUM evacuation:
        q = istd .* psq - (mu*istd) .* rq + bq         (DVE, 3 ops)
        k = psk - mu .* rk                             (DVE STT, 1 op;
          the istd_k factor rides for free in the exp's per-partition scale)
        v = istd .* (psv - mu .* rv)                   (DVE, 2 ops;
          the V bias is exact via softmax-rows-sum-to-1 and is folded into a
          host-precomputed output bias added in the output-GEMM epilogue)
    where rq/rk/rv are host-precomputed column sums of the LN-folded QKV
    weight and mu/istd come from on-device bn_stats over the natural copy,
    transposed via one tiny [128,128] XBAR per superblock and broadcast with
    GpSimd partition_broadcast.
  - Attention per (batch, head) fully transposed as v3, but the softmax
    rowsum matmul uses an all-ones [128,128] stationary (M=128, not M=1):
    the PSUM result is the rowsum pre-broadcast to all partitions, so the
    normalize is just reciprocal_approx_fast + one tensor_tensor multiply.
    No ScalarE copies and -- critically -- NO GpSimd broadcasts in
    attention, so the AllToAll collectives are not stuck behind attention
    work in the GpSimd queue: A2A(b0) now fires at the start of phase B and
    hides under attention(b1).
  - Output GEMM runs cc-outer over nb-pairs so a stationary cf chunk is
    reused across two 512-wide streams; outG(b0) interleaves into the tail
    of attention(b1) and covers A2A(b1).
"""

import sys

if "/opt/trn_rl_repo" not in sys.path:
    sys.path.insert(0, "/opt/trn_rl_repo")

# --- shim antenv.axon_hooks (missing in this image) so trace=True can NTFF-profile ---
import types, ctypes, contextlib


def _make_ntff_hook(so_path="/opt/axon/libaxon_pjrt.so"):
    try:
        lib = ctypes.CDLL(so_path)
    except OSError:
        return None
    if not hasattr(lib, "axon_start_nrt_profile"):
        return None
    lib.axon_start_nrt_profile.argtypes = [ctypes.POINTER(ctypes.c_int64), ctypes.c_size_t]
    lib.axon_start_nrt_profile.restype = ctypes.c_int64
    lib.axon_stop_nrt_profile.argtypes = [ctypes.c_char_p]
    lib.axon_stop_nrt_profile.restype = ctypes.c_int64

    @contextlib.contextmanager
    def _hook(output_dir, device_ids):
        import jax

        jax.devices()
        if device_ids:
            ids = (ctypes.c_int64 * len(device_ids))(*device_ids)
            rc = lib.axon_start_nrt_profile(ids, len(device_ids))
        else:
            rc = lib.axon_start_nrt_profile(None, 0)
        if rc != 0:
            raise RuntimeError(f"axon_start_nrt_profile rc={rc}")
        try:
            yield
        finally:
            n = lib.axon_stop_nrt_profile(str(output_dir).encode())
            if n < 0:
                raise RuntimeError(f"axon_stop_nrt_profile rc={n}")

    return _hook


if "antenv.axon_hooks" not in sys.modules:
    _m = types.ModuleType("antenv.axon_hooks")
    _m.get_axon_ntff_profile_hook = lambda: _make_ntff_hook()
    sys.modules["antenv.axon_hooks"] = _m
# --- end shim ---

import numpy as np
import ml_dtypes  # noqa: F401  (bf16 numpy dtype registration)

from concourse import bacc, tile, mybir
from concourse.masks import make_upper_triangular

B, S, HID = 2, 2048, 2048
HEADS = 16
HD = 128
T = B * S
N_CORES = 8
HPC = HEADS // N_CORES  # 2 heads per core
EPS = 1e-6
SCALE = 1.0 / float(np.sqrt(HD))

F32 = mybir.dt.float32
BF16 = mybir.dt.bfloat16

SB = 512  # tokens per LN/QKV superblock
N_SB = T // SB  # 8
N_CC = HID // 128  # 16 contraction chunks
TOK_SHARD = S // N_CORES  # 256 tokens per (batch, core) after A2A
QC = 512  # attention q-chunk width
MUL = mybir.AluOpType.mult
SUB = mybir.AluOpType.subtract
ADD = mybir.AluOpType.add


def _build(apply_mask: bool):
    nc = bacc.Bacc("TRN2", target_bir_lowering=False, debug=False, num_devices=N_CORES)

    xt = nc.dram_tensor("xt", [HID, T], BF16, kind="ExternalInput").ap()
    xn = nc.dram_tensor("xn", [T, HID], BF16, kind="ExternalInput").ap()
    wq = nc.dram_tensor("qkvw", [HID, 3 * HPC * HD], BF16, kind="ExternalInput").ap()
    rqk = nc.dram_tensor("rqk", [128, 4], F32, kind="ExternalInput").ap()
    bqd = nc.dram_tensor("bq", [128, HPC], F32, kind="ExternalInput").ap()
    rvd = nc.dram_tensor("rv", [1, HPC * HD], BF16, kind="ExternalInput").ap()
    obd = nc.dram_tensor("obias", [1, HID], BF16, kind="ExternalInput").ap()
    owt = nc.dram_tensor("ow", [HID, HID], BF16, kind="ExternalInput").ap()
    out = nc.dram_tensor("out", [B * TOK_SHARD, HID], F32, kind="ExternalOutput").ap()
    if apply_mask:
        imask = nc.dram_tensor("imask", [128, B * (S // 128)], F32, kind="ExternalInput").ap()

    cc_in = [nc.dram_tensor(f"cc_in{b}", [N_CORES, HPC * HD, TOK_SHARD], BF16).ap() for b in range(B)]
    cc_out = [nc.dram_tensor(f"cc_out{b}", [N_CORES, HPC * HD, TOK_SHARD], BF16).ap() for b in range(B)]

    with tile.TileContext(nc) as tc:
        with tc.tile_pool(name="persist", bufs=1) as pers:
            ones128 = pers.tile([128, 128], BF16)
            nc.gpsimd.memset(ones128[:], 1.0)
            eps_t = pers.tile([128, 1], F32)
            nc.gpsimd.memset(eps_t[:], EPS)
            trif = pers.tile([128, 128], F32)
            make_upper_triangular(nc, trif[:], val=1.0, diag=True)
            tri01 = pers.tile([128, 128], BF16)
            nc.vector.tensor_copy(tri01[:], trif[:])
            rqk_sb = pers.tile([128, 4], F32)
            nc.scalar.dma_start(out=rqk_sb[:], in_=rqk[:])
            bq_sb = pers.tile([128, HPC], F32)
            nc.scalar.dma_start(out=bq_sb[:], in_=bqd[:])
            rv_row = pers.tile([1, HPC * HD], BF16)
            nc.scalar.dma_start(out=rv_row[:], in_=rvd[:])
            rv_bc = pers.tile([128, HPC * HD], BF16)
            nc.gpsimd.partition_broadcast(rv_bc[:], rv_row[:])
            ob_row = pers.tile([1, HID], BF16)
            nc.scalar.dma_start(out=ob_row[:], in_=obd[:])
            ob_bc = pers.tile([128, HID], BF16)
            nc.gpsimd.partition_broadcast(ob_bc[:], ob_row[:])
            if apply_mask:
                msk = pers.tile([128, B * (S // 128)], F32)
                nc.scalar.dma_start(out=msk[:], in_=imask[:])

            qT = pers.tile([128, HPC, T], BF16)  # [d, head, tok]
            kT = pers.tile([128, HPC, T], BF16)
            v_sb = pers.tile([128, T // 128, HPC * HD], BF16)  # [tok128, blk, hcol]
            sistd = pers.tile([128, T // 128], F32)  # SCALE * istd per token-block
            istd_n = pers.tile([128, T // 128], F32)  # 1/sd, natural layout
            c1_n = pers.tile([128, T // 128], F32)  # mu/sd, natural layout

            # ---------- attention emitter (transposed, sw-pipelined) ----------
            def attn_qc(b, qc, ps_mm, ps_ctx, ps_rs, ppT, prs, ctxT):
                nkb = 4 * qc + 4
                ctx_ps = [ps_ctx.tile([128, QC], F32, tag="ctx", name=f"ctx{h}") for h in range(HPC)]
                rs_ps = [ps_rs.tile([128, QC], F32, tag="rs", name=f"rs{h}") for h in range(HPC)]
                kbs = list(range(nkb - 1, -1, -1))
                pend = {}

                def emit_sc(kb):
                    c0 = max(0, (kb - 4 * qc) * 128)
                    w = QC - c0
                    for h in range(HPC):
                        sc = ps_mm.tile([128, QC], F32, tag="mm", name="sc")
                        nc.tensor.matmul(
                            sc[:, :w],
                            kT[:, h, b * S + kb * 128 : b * S + kb * 128 + 128],
                            qT[:, h, b * S + qc * QC + c0 : b * S + qc * QC + c0 + w],
                            start=True,
                            stop=True,
                        )
                        pt = ppT.tile([128, QC], BF16, tag="pt", name="pt")
                        bias = msk[:, b * 16 + kb : b * 16 + kb + 1] if apply_mask else 0.0
                        nc.scalar.activation(
                            pt[:, :w], sc[:, :w], mybir.ActivationFunctionType.Exp,
                            scale=sistd[:, b * 16 + kb : b * 16 + kb + 1], bias=bias,
                        )
                        if kb >= 4 * qc:  # causal diagonal block
                            nc.vector.tensor_mul(pt[:, 0:128], pt[:, 0:128], tri01[:])
                        pend[(h, kb)] = (pt, c0, w)

                def emit_consume(kb):
                    for h in range(HPC):
                        pt, c0, w = pend.pop((h, kb))
                        nc.tensor.matmul(
                            rs_ps[h][:, c0:QC], ones128[:], pt[:, :w],
                            start=(kb == kbs[0]), stop=(kb == 0),
                        )
                        nc.tensor.matmul(
                            ctx_ps[h][:, c0:QC],
                            v_sb[:, b * 16 + kb, h * HD : (h + 1) * HD],
                            pt[:, :w],
                            start=(kb == kbs[0]), stop=(kb == 0),
                        )

                for i, kb in enumerate(kbs):
                    emit_sc(kb)
                    if i > 0:
                        emit_consume(kbs[i - 1])
                emit_consume(kbs[-1])

                for h in range(HPC):
                    rsi = prs.tile([128, QC], F32, tag="rsi", name="rsi")
                    nc.vector.reciprocal_approx_fast(out=rsi[:], in_=rs_ps[h][:])
                    nc.vector.tensor_mul(
                        ctxT[:, h, qc * QC : (qc + 1) * QC], ctx_ps[h][:], rsi[:]
                    )

            def ship_ctx(b, ctxT):
                for j in range(N_CORES):
                    nc.sync.dma_start(
                        out=cc_in[b][j].rearrange("(h d) w -> d h w", d=128),
                        in_=ctxT[:, :, j * TOK_SHARD : (j + 1) * TOK_SHARD],
                    )
                nc.gpsimd.collective_compute(
                    "AllToAll",
                    mybir.AluOpType.bypass,
                    replica_groups=[list(range(N_CORES))],
                    ins=[cc_in[b][:]],
                    outs=[cc_out[b][:]],
                )

            # ---------------- Phase A + attention(b0) interleaved ----------------
            with (
                tc.tile_pool(name="pb_pT", bufs=6) as ppT,
                tc.tile_pool(name="pb_cT", bufs=2) as pcT,
                tc.tile_pool(name="pb_rs_sb", bufs=2) as prs,
                tc.tile_pool(name="ps_mm", bufs=4, space="PSUM") as ps_mm,
                tc.tile_pool(name="ps_ctx", bufs=2, space="PSUM") as ps_ctx,
                tc.tile_pool(name="ps_rs", bufs=2, space="PSUM") as ps_rs,
            ):
                ctxT0 = pcT.tile([128, HPC, S], BF16, tag="ctxT", name="ctxT0")
                with (
                    tc.tile_pool(name="pa_w", bufs=1) as paw,
                    tc.tile_pool(name="pa_xt", bufs=3) as pxt,
                    tc.tile_pool(name="pa_xn", bufs=6) as pxn,
                    tc.tile_pool(name="pa_st", bufs=6) as pst,
                    tc.tile_pool(name="pa_A", bufs=2) as pA,
                    tc.tile_pool(name="pa_bc", bufs=3) as pbc,
                    tc.tile_pool(name="pa_ev", bufs=6) as pe,
                ):
                    w_sb = paw.tile([128, N_CC, 3 * HPC * HD], BF16)
                    for g in range(4):
                        nc.scalar.dma_start(
                            out=w_sb[:, g * 4 : (g + 1) * 4, :],
                            in_=wq[g * 512 : (g + 1) * 512, :].rearrange(
                                "(c p) f -> p c f", p=128
                            ),
                        )

                    def stats_sb(sb):
                        """bn_stats on natural x -> mu/istd/c1.  Each stat is
                        replicated across 128 columns (TS with the all-ones
                        tile), so one XBAR transpose of [128, 3*SB] yields the
                        partition-broadcast [128, SB] tiles directly:
                        bcast[:, s*4+tb, :] = stat_s(block tb) in every row."""
                        srep = pA.tile([128, 3 * SB], BF16, tag="A", name="srep")
                        for tb in range(4):
                            r0 = sb * SB + tb * 128
                            x_t = pxn.tile([128, HID], BF16, tag="xn", name="x_t")
                            nc.gpsimd.dma_start(out=x_t[:], in_=xn[r0 : r0 + 128, :])
                            bn = pst.tile([128, 4, 6], F32, tag="bn", name="bn")
                            for c4 in range(4):
                                nc.vector.bn_stats(bn[:, c4, :], x_t[:, c4 * 512 : (c4 + 1) * 512])
                            mv = pst.tile([128, 2], F32, tag="mv", name="mv")
                            nc.vector.bn_aggr(mv[:], bn[:])
                            sd = pst.tile([128, 1], F32, tag="sd", name="sd")
                            nc.scalar.activation(
                                sd[:], mv[:, 1:2], mybir.ActivationFunctionType.Sqrt, bias=eps_t[:]
                            )
                            istd = pst.tile([128, 1], F32, tag="istd", name="istd")
                            nc.vector.reciprocal_approx_fast(out=istd[:], in_=sd[:])
                            blk = sb * 4 + tb
                            nc.vector.tensor_scalar(
                                out=sistd[:, blk : blk + 1], in0=istd[:],
                                scalar1=SCALE, scalar2=None, op0=MUL,
                            )
                            nc.vector.tensor_copy(istd_n[:, blk : blk + 1], istd[:])
                            nc.vector.tensor_scalar(
                                out=c1_n[:, blk : blk + 1], in0=mv[:, 0:1],
                                scalar1=istd[:], scalar2=None, op0=MUL,
                            )
                            cs = slice(0 * SB + tb * 128, 0 * SB + (tb + 1) * 128)
                            nc.gpsimd.tensor_scalar(
                                out=srep[:, cs], in0=ones128[:],
                                scalar1=mv[:, 0:1], scalar2=None, op0=MUL,
                            )
                            cs = slice(1 * SB + tb * 128, 1 * SB + (tb + 1) * 128)
                            nc.gpsimd.tensor_scalar(
                                out=srep[:, cs], in0=ones128[:],
                                scalar1=istd[:], scalar2=None, op0=MUL,
                            )
                            cs = slice(2 * SB + tb * 128, 2 * SB + (tb + 1) * 128)
                            nc.gpsimd.tensor_scalar(
                                out=srep[:, cs], in0=ones128[:],
                                scalar1=c1_n[:, blk : blk + 1], scalar2=None, op0=MUL,
                            )
                        bc = pbc.tile([128, 12, 128], BF16, tag="bc", name="bc")
                        nc.scalar.dma_start_transpose(out=bc[:], in_=srep[:])
                        flat = bc[:].rearrange("p a b -> p (a b)")
                        mu_bc = flat[:, 0 * SB : 1 * SB]
                        istd_bc = flat[:, 1 * SB : 2 * SB]
                        c1_bc = flat[:, 2 * SB : 3 * SB]
                        return mu_bc, istd_bc, c1_bc

                    def load_xt(sb):
                        """Split into 4-cc groups so the first chain matmuls can
                        start as soon as the first 0.5MB lands."""
                        col0 = sb * SB
                        xt_t = pxt.tile([128, N_CC, SB], BF16, tag="xt", name="xt_t")
                        for g in range(4):
                            nc.sync.dma_start(
                                out=xt_t[:, g * 4 : (g + 1) * 4, :],
                                in_=xt[g * 512 : (g + 1) * 512, col0 : col0 + SB].rearrange(
                                    "(c p) t -> p c t", p=128
                                ),
                            )
                        return xt_t

                    def qkv_sb(sb, bcs, xt_t):
                        mu_bc, istd_bc, c1_bc = bcs
                        col0 = sb * SB
                        for h in range(HPC):
                            psq = ps_mm.tile([128, SB], F32, tag="mm", name="psq")
                            for cc in range(N_CC):
                                nc.tensor.matmul(
                                    psq[:],
                                    w_sb[:, cc, h * HD : (h + 1) * HD],
                                    xt_t[:, cc, :],
                                    start=(cc == 0),
                                    stop=(cc == N_CC - 1),
                                )
                            t1 = pe.tile([128, SB], BF16, tag="ev", name="t1")
                            nc.vector.tensor_mul(t1[:], psq[:], istd_bc)
                            # t2 = c1*rq - bq on ScalarE (bq negated on host)
                            t2 = pe.tile([128, SB], BF16, tag="ev", name="t2")
                            nc.scalar.activation(
                                t2[:], c1_bc, mybir.ActivationFunctionType.Identity,
                                scale=rqk_sb[:, h : h + 1], bias=bq_sb[:, h : h + 1],
                            )
                            nc.vector.tensor_sub(qT[:, h, col0 : col0 + SB], t1[:], t2[:])

                            psk = ps_mm.tile([128, SB], F32, tag="mm", name="psk")
                            for cc in range(N_CC):
                                nc.tensor.matmul(
                                    psk[:],
                                    w_sb[:, cc, HPC * HD + h * HD : HPC * HD + (h + 1) * HD],
                                    xt_t[:, cc, :],
                                    start=(cc == 0),
                                    stop=(cc == N_CC - 1),
                                )
                            # k = psk - mu*rk  (rk negated on host; istd_k folded
                            # into the exp scale)
                            nc.vector.scalar_tensor_tensor(
                                out=kT[:, h, col0 : col0 + SB],
                                in0=mu_bc, scalar=rqk_sb[:, 2 + h : 3 + h], in1=psk[:],
                                op0=MUL, op1=ADD,
                            )
                        for tb2 in range(4):
                            psv = ps_mm.tile([128, HPC * HD], F32, tag="mm", name="psv")
                            for cc in range(N_CC):
                                nc.tensor.matmul(
                                    psv[:],
                                    xt_t[:, cc, tb2 * 128 : (tb2 + 1) * 128],
                                    w_sb[:, cc, 2 * HPC * HD :],
                                    start=(cc == 0),
                                    stop=(cc == N_CC - 1),
                                )
                            blk = sb * 4 + tb2
                            va = pe.tile([128, HPC * HD], BF16, tag="ev", name="va")
                            nc.scalar.activation(
                                va[:], psv[:], mybir.ActivationFunctionType.Identity,
                                scale=istd_n[:, blk : blk + 1],
                            )
                            # v = va - c1*rv  (rv negated on host)
                            nc.vector.scalar_tensor_tensor(
                                out=v_sb[:, blk, :],
                                in0=rv_bc[:], scalar=c1_n[:, blk : blk + 1], in1=va[:],
                                op0=MUL, op1=ADD,
                            )

                    bcs = [None] * N_SB
                    xts = [None] * N_SB
                    bcs[0] = stats_sb(0)
                    xts[0] = load_xt(0)
                    bcs[1] = stats_sb(1)
                    xts[1] = load_xt(1)
                    qkv_sb(0, bcs[0], xts[0])
                    bcs[2] = stats_sb(2)
                    xts[2] = load_xt(2)
                    qkv_sb(1, bcs[1], xts[1])
                    bcs[3] = stats_sb(3)
                    xts[3] = load_xt(3)
                    qkv_sb(2, bcs[2], xts[2])
                    bcs[4] = stats_sb(4)
                    xts[4] = load_xt(4)
                    qkv_sb(3, bcs[3], xts[3])
                    bcs[5] = stats_sb(5)
                    xts[5] = load_xt(5)
                    qkv_sb(4, bcs[4], xts[4])
                    attn_qc(0, 0, ps_mm, ps_ctx, ps_rs, ppT, prs, ctxT0)
                    bcs[6] = stats_sb(6)
                    xts[6] = load_xt(6)
                    qkv_sb(5, bcs[5], xts[5])
                    attn_qc(0, 1, ps_mm, ps_ctx, ps_rs, ppT, prs, ctxT0)
                    bcs[7] = stats_sb(7)
                    xts[7] = load_xt(7)
                    qkv_sb(6, bcs[6], xts[6])
                    attn_qc(0, 2, ps_mm, ps_ctx, ps_rs, ppT, prs, ctxT0)
                    qkv_sb(7, bcs[7], xts[7])
                    attn_qc(0, 3, ps_mm, ps_ctx, ps_rs, ppT, prs, ctxT0)
                    ship_ctx(0, ctxT0)

                # ---------- Phase B: attention(b1) + output GEMMs ----------
                with (
                    tc.tile_pool(name="pb_ow", bufs=1) as pow_,
                    tc.tile_pool(name="pb_cf", bufs=2) as pcf,
                    tc.tile_pool(name="pb_o", bufs=2) as po,
                ):
                    ow_sb = pow_.tile([128, N_CC, HID], BF16)
                    for g in range(4):
                        nc.scalar.dma_start(
                            out=ow_sb[:, g * 4 : (g + 1) * 4, :],
                            in_=owt[g * 512 : (g + 1) * 512, :].rearrange(
                                "(c p) f -> p c f", p=128
                            ),
                        )
                    def load_cf(b):
                        cf = pcf.tile([128, N_CC, TOK_SHARD], BF16, tag="cf", name=f"cf{b}")
                        src = cc_out[b].rearrange("j (h d) w -> d (j h) w", d=128)
                        nc.sync.dma_start(out=cf[:, 0:8, :], in_=src[:, 0:8, :])
                        nc.sync.dma_start(out=cf[:, 8:16, :], in_=src[:, 8:16, :])
                        return cf

                    cf0 = load_cf(0)

                    def outg_tb(b, cf, tb):
                        o_t = po.tile([128, HID], F32, tag="o", name="o_t")
                        for half in range(2):
                            pso = [
                                ps_mm.tile([128, 512], F32, tag="mm", name=f"pso{nb}")
                                for nb in (2 * half, 2 * half + 1)
                            ]
                            for cc in range(N_CC):
                                for i, nb in enumerate((2 * half, 2 * half + 1)):
                                    nc.tensor.matmul(
                                        pso[i][:],
                                        cf[:, cc, tb * 128 : (tb + 1) * 128],
                                        ow_sb[:, cc, nb * 512 : (nb + 1) * 512],
                                        start=(cc == 0),
                                        stop=(cc == N_CC - 1),
                                    )
                            for i, nb in enumerate((2 * half, 2 * half + 1)):
                                nc.vector.tensor_add(
                                    o_t[:, nb * 512 : (nb + 1) * 512], pso[i][:],
                                    ob_bc[:, nb * 512 : (nb + 1) * 512],
                                )
                        nc.sync.dma_start(
                            out=out[b * TOK_SHARD + tb * 128 : b * TOK_SHARD + (tb + 1) * 128, :],
                            in_=o_t[:],
                        )

                    ctxT1 = pcT.tile([128, HPC, S], BF16, tag="ctxT", name="ctxT1")
                    attn_qc(1, 0, ps_mm, ps_ctx, ps_rs, ppT, prs, ctxT1)
                    attn_qc(1, 1, ps_mm, ps_ctx, ps_rs, ppT, prs, ctxT1)
                    attn_qc(1, 2, ps_mm, ps_ctx, ps_rs, ppT, prs, ctxT1)
                    attn_qc(1, 3, ps_mm, ps_ctx, ps_rs, ppT, prs, ctxT1)
                    ship_ctx(1, ctxT1)
                    # outG(b0) fills the PE while A2A(b1) is in flight
                    outg_tb(0, cf0, 0)
                    cf1 = load_cf(1)
                    outg_tb(0, cf0, 1)
                    outg_tb(1, cf1, 0)
                    outg_tb(1, cf1, 1)

    nc.compile()
    return nc


_CACHE = {}


def _get_nc(apply_mask: bool):
    if apply_mask not in _CACHE:
        _CACHE[apply_mask] = _build(apply_mask)
    return _CACHE[apply_mask]


def _prep_in_maps(input, input_mask, norm_w, norm_b, attn_qkvw, attn_qkvb, attn_ow):
    bf16 = ml_dtypes.bfloat16
    x = np.asarray(input, dtype=np.float32).reshape(T, HID)
    w = np.asarray(attn_qkvw, dtype=np.float32)
    nw = np.asarray(norm_w, dtype=np.float32)
    nb = np.asarray(norm_b, dtype=np.float32)
    qb_ = np.asarray(attn_qkvb, dtype=np.float32)
    ow_f = np.asarray(attn_ow, dtype=np.float32)
    ow = np.ascontiguousarray(ow_f.astype(bf16))
    mask = np.asarray(input_mask, dtype=np.float32).reshape(B, S)

    xt = np.ascontiguousarray(x.T.astype(bf16))  # [HID, T]
    xn = np.ascontiguousarray(x.astype(bf16))  # [T, HID]

    w_eff = nw[:, None] * w  # fold LN gamma into QKV weight
    b_eff = nb @ w + qb_  # fold LN beta into QKV bias
    colsum = w_eff.sum(axis=0)  # [3*HID]
    obias_full = np.ascontiguousarray(
        (b_eff[2 * HID :] @ ow_f).reshape(1, HID).astype(bf16)
    )

    apply_mask = bool(np.any(mask != 0.0))
    if apply_mask:
        # per-key layout: [128 partitions (k within block), B * 16 key-blocks]
        mprep = np.ascontiguousarray(
            mask.reshape(B, S // 128, 128).transpose(2, 0, 1).reshape(128, B * (S // 128))
        )
    in_maps = []
    for i in range(N_CORES):
        cols = []
        for part in range(3):  # q, k, v column shards for this core's heads
            c0 = part * HID + i * HPC * HD
            cols.append(w_eff[:, c0 : c0 + HPC * HD])
        wqkv_i = np.ascontiguousarray(np.concatenate(cols, axis=1).astype(bf16))

        q0 = i * HPC * HD
        k0 = HID + i * HPC * HD
        v0 = 2 * HID + i * HPC * HD
        rqk_i = np.ascontiguousarray(
            np.stack(
                [
                    colsum[q0 : q0 + HD],
                    colsum[q0 + HD : q0 + 2 * HD],
                    -colsum[k0 : k0 + HD],
                    -colsum[k0 + HD : k0 + 2 * HD],
                ],
                axis=1,
            ).astype(np.float32)
        )
        # negated: the ScalarE Identity epilogue computes c1*rq + (-bq)
        bq_i = np.ascontiguousarray(
            np.stack(
                [-b_eff[q0 : q0 + HD], -b_eff[q0 + HD : q0 + 2 * HD]], axis=1
            ).astype(np.float32)
        )
        rv_i = np.ascontiguousarray(
            (-colsum[v0 : v0 + HPC * HD]).reshape(1, HPC * HD).astype(bf16)
        )
        m = {
            "xt": xt,
            "xn": xn,
            "qkvw": wqkv_i,
            "rqk": rqk_i,
            "bq": bq_i,
            "rv": rv_i,
            "obias": obias_full,
            "ow": ow,
        }
        if apply_mask:
            m["imask"] = mprep
        in_maps.append(m)
    return in_maps, apply_mask


def _run(inputs: dict, trace: bool = False):
    from concourse.bass_utils import run_bass_kernel_spmd

    in_maps, apply_mask = _prep_in_maps(**inputs)
    nc = _get_nc(apply_mask)
    res = run_bass_kernel_spmd(nc, in_maps, list(range(N_CORES)), trace=trace)
    out = np.empty((B, S, HID), dtype=np.float32)
    for j in range(N_CORES):
        o = res.results[j]["out"]
        for b in range(B):
            out[b, j * TOK_SHARD : (j + 1) * TOK_SHARD] = o[b * TOK_SHARD : (b + 1) * TOK_SHARD]
    return out, res


def kernel(**inputs) -> np.ndarray:
    out, _ = _run(inputs, trace=False)
    return out


# revision 22
# speedup vs baseline: 1.2188x; 1.2188x over previous
"""Tensor-parallel DeepSpeed-style self-attention block on 8 TRN2 NeuronCores.

v4 strategy (fold LN into GEMM epilogues; kill startup + queue serialization):
  - Host pre-transposes the input to xT (bf16) and also passes a natural
    bf16 copy for LayerNorm statistics.  The QKV GEMM runs directly on the
    RAW xT -- no z pass, no on-device zT transposes -- so the first matmul
    fires as soon as the first weight/xT chunks land (~10us vs ~67us).
  - LN is applied algebraically in the PSUM evacuation:
        q = istd .* psq - (mu*istd) .* rq + bq         (DVE, 3 ops)
        k = psk - mu .* rk                             (DVE STT, 1 op;
          the istd_k factor rides for free in the exp's per-partition scale)
        v = istd .* (psv - mu .* rv)                   (DVE, 2 ops;
          the V bias is exact via softmax-rows-sum-to-1 and is folded into a
          host-precomputed output bias added in the output-GEMM epilogue)
    where rq/rk/rv are host-precomputed column sums of the LN-folded QKV
    weight and mu/istd come from on-device bn_stats over the natural copy,
    transposed via one tiny [128,128] XBAR per superblock and broadcast with
    GpSimd partition_broadcast.
  - Attention per (batch, head) fully transposed as v3, but the softmax
    rowsum matmul uses an all-ones [128,128] stationary (M=128, not M=1):
    the PSUM result is the rowsum pre-broadcast to all partitions, so the
    normalize is just reciprocal_approx_fast + one tensor_tensor multiply.
    No ScalarE copies and -- critically -- NO GpSimd broadcasts in
    attention, so the AllToAll collectives are not stuck behind attention
    work in the GpSimd queue: A2A(b0) now fires at the start of phase B and
    hides under attention(b1).
  - Output GEMM runs cc-outer over nb-pairs so a stationary cf chunk is
    reused across two 512-wide streams; outG(b0) interleaves into the tail
    of attention(b1) and covers A2A(b1).
"""

import sys

if "/opt/trn_rl_repo" not in sys.path:
    sys.path.insert(0, "/opt/trn_rl_repo")

# --- shim antenv.axon_hooks (missing in this image) so trace=True can NTFF-profile ---
import types, ctypes, contextlib


def _make_ntff_hook(so_path="/opt/axon/libaxon_pjrt.so"):
    try:
        lib = ctypes.CDLL(so_path)
    except OSError:
        return None
    if not hasattr(lib, "axon_start_nrt_profile"):
        return None
    lib.axon_start_nrt_profile.argtypes = [ctypes.POINTER(ctypes.c_int64), ctypes.c_size_t]
    lib.axon_start_nrt_profile.restype = ctypes.c_int64
    lib.axon_stop_nrt_profile.argtypes = [ctypes.c_char_p]
    lib.axon_stop_nrt_profile.restype = ctypes.c_int64

    @contextlib.contextmanager
    def _hook(output_dir, device_ids):
        import jax

        jax.devices()
        if device_ids:
            ids = (ctypes.c_int64 * len(device_ids))(*device_ids)
            rc = lib.axon_start_nrt_profile(ids, len(device_ids))
        else:
            rc = lib.axon_start_nrt_profile(None, 0)
        if rc != 0:
            raise RuntimeError(f"axon_start_nrt_profile rc={rc}")
        try:
            yield
        finally:
            n = lib.axon_stop_nrt_profile(str(output_dir).encode())
            if n < 0:
                raise RuntimeError(f"axon_stop_nrt_profile rc={n}")

    return _hook


if "antenv.axon_hooks" not in sys.modules:
    _m = types.ModuleType("antenv.axon_hooks")
    _m.get_axon_ntff_profile_hook = lambda: _make_ntff_hook()
    sys.modules["antenv.axon_hooks"] = _m
# --- end shim ---

import numpy as np
import ml_dtypes  # noqa: F401  (bf16 numpy dtype registration)

from concourse import bacc, tile, mybir
from concourse.masks import make_upper_triangular

B, S, HID = 2, 2048, 2048
HEADS = 16
HD = 128
T = B * S
N_CORES = 8
HPC = HEADS // N_CORES  # 2 heads per core
EPS = 1e-6
SCALE = 1.0 / float(np.sqrt(HD))

F32 = mybir.dt.float32
BF16 = mybir.dt.bfloat16

SB = 512  # tokens per LN/QKV superblock
N_SB = T // SB  # 8
N_CC = HID // 128  # 16 contraction chunks
TOK_SHARD = S // N_CORES  # 256 tokens per (batch, core) after A2A
QC = 512  # attention q-chunk width
MUL = mybir.AluOpType.mult
SUB = mybir.AluOpType.subtract
ADD = mybir.AluOpType.add


def _build(apply_mask: bool):
    nc = bacc.Bacc("TRN2", target_bir_lowering=False, debug=False, num_devices=N_CORES)

    xt = nc.dram_tensor("xt", [HID, T], BF16, kind="ExternalInput").ap()
    xn = nc.dram_tensor("xn", [T, HID], BF16, kind="ExternalInput").ap()
    wq = nc.dram_tensor("qkvw", [HID, 3 * HPC * HD], BF16, kind="ExternalInput").ap()
    rqk = nc.dram_tensor("rqk", [128, 4], F32, kind="ExternalInput").ap()
    bqd = nc.dram_tensor("bq", [128, HPC], F32, kind="ExternalInput").ap()
    rvd = nc.dram_tensor("rv", [1, HPC * HD], BF16, kind="ExternalInput").ap()
    obd = nc.dram_tensor("obias", [1, HID], BF16, kind="ExternalInput").ap()
    owt = nc.dram_tensor("ow", [HID, HID], BF16, kind="ExternalInput").ap()
    out = nc.dram_tensor("out", [B * TOK_SHARD, HID], F32, kind="ExternalOutput").ap()
    if apply_mask:
        imask = nc.dram_tensor("imask", [128, B * (S // 128)], F32, kind="ExternalInput").ap()

    cc_in = [nc.dram_tensor(f"cc_in{b}", [N_CORES, HPC * HD, TOK_SHARD], BF16).ap() for b in range(B)]
    cc_out = [nc.dram_tensor(f"cc_out{b}", [N_CORES, HPC * HD, TOK_SHARD], BF16).ap() for b in range(B)]

    with tile.TileContext(nc) as tc:
        with tc.tile_pool(name="persist", bufs=1) as pers:
            ones128 = pers.tile([128, 128], BF16)
            nc.gpsimd.memset(ones128[:], 1.0)
            eps_t = pers.tile([128, 1], F32)
            nc.gpsimd.memset(eps_t[:], EPS)
            trif = pers.tile([128, 128], F32)
            make_upper_triangular(nc, trif[:], val=1.0, diag=True)
            tri01 = pers.tile([128, 128], BF16)
            nc.vector.tensor_copy(tri01[:], trif[:])
            rqk_sb = pers.tile([128, 4], F32)
            nc.scalar.dma_start(out=rqk_sb[:], in_=rqk[:])
            bq_sb = pers.tile([128, HPC], F32)
            nc.scalar.dma_start(out=bq_sb[:], in_=bqd[:])
            rv_row = pers.tile([1, HPC * HD], BF16)
            nc.scalar.dma_start(out=rv_row[:], in_=rvd[:])
            rv_bc = pers.tile([128, HPC * HD], BF16)
            nc.gpsimd.partition_broadcast(rv_bc[:], rv_row[:])
            ob_row = pers.tile([1, HID], BF16)
            nc.scalar.dma_start(out=ob_row[:], in_=obd[:])
            ob_bc = pers.tile([128, HID], BF16)
            nc.gpsimd.partition_broadcast(ob_bc[:], ob_row[:])
            if apply_mask:
                msk = pers.tile([128, B * (S // 128)], F32)
                nc.scalar.dma_start(out=msk[:], in_=imask[:])

            qT = pers.tile([128, HPC, T], BF16)  # [d, head, tok]
            kT = pers.tile([128, HPC, T], BF16)
            v_sb = pers.tile([128, T // 128, HPC * HD], BF16)  # [tok128, blk, hcol]
            sistd = pers.tile([128, T // 128], F32)  # SCALE * istd per token-block
            istd_n = pers.tile([128, T // 128], F32)  # 1/sd, natural layout
            c1_n = pers.tile([128, T // 128], F32)  # mu/sd, natural layout

            # ---------- attention emitter (transposed, sw-pipelined) ----------
            def attn_qc(b, qc, ps_mm, ps_ctx, ps_rs, ppT, prs, ctxT):
                nkb = 4 * qc + 4
                ctx_ps = [ps_ctx.tile([128, QC], F32, tag="ctx", name=f"ctx{h}") for h in range(HPC)]
                rs_ps = [ps_rs.tile([128, QC], F32, tag="rs", name=f"rs{h}") for h in range(HPC)]
                kbs = list(range(nkb - 1, -1, -1))
                pend = {}

                def emit_sc(kb):
                    c0 = max(0, (kb - 4 * qc) * 128)
                    w = QC - c0
                    for h in range(HPC):
                        sc = ps_mm.tile([128, QC], F32, tag="mm", name="sc")
                        nc.tensor.matmul(
                            sc[:, :w],
                            kT[:, h, b * S + kb * 128 : b * S + kb * 128 + 128],
                            qT[:, h, b * S + qc * QC + c0 : b * S + qc * QC + c0 + w],
                            start=True,
                            stop=True,
                        )
                        pt = ppT.tile([128, QC], BF16, tag="pt", name="pt")
                        bias = msk[:, b * 16 + kb : b * 16 + kb + 1] if apply_mask else 0.0
                        nc.scalar.activation(
                            pt[:, :w], sc[:, :w], mybir.ActivationFunctionType.Exp,
                            scale=sistd[:, b * 16 + kb : b * 16 + kb + 1], bias=bias,
                        )
                        if kb >= 4 * qc:  # causal diagonal block
                            nc.vector.tensor_mul(pt[:, 0:128], pt[:, 0:128], tri01[:])
                        pend[(h, kb)] = (pt, c0, w)

                def emit_consume(kb):
                    for h in range(HPC):
                        pt, c0, w = pend.pop((h, kb))
                        nc.tensor.matmul(
                            rs_ps[h][:, c0:QC], ones128[:], pt[:, :w],
                            start=(kb == kbs[0]), stop=(kb == 0),
                        )
                        nc.tensor.matmul(
                            ctx_ps[h][:, c0:QC],
                            v_sb[:, b * 16 + kb, h * HD : (h + 1) * HD],
                            pt[:, :w],
                            start=(kb == kbs[0]), stop=(kb == 0),
                        )

                for i, kb in enumerate(kbs):
                    emit_sc(kb)
                    if i > 0:
                        emit_consume(kbs[i - 1])
                emit_consume(kbs[-1])

                for h in range(HPC):
                    rsi = prs.tile([128, QC], F32, tag="rsi", name="rsi")
                    nc.vector.reciprocal_approx_fast(out=rsi[:], in_=rs_ps[h][:])
                    nc.vector.tensor_mul(
                        ctxT[:, h, qc * QC : (qc + 1) * QC], ctx_ps[h][:], rsi[:]
                    )

            def ship_ctx(b, ctxT):
                for j in range(N_CORES):
                    nc.sync.dma_start(
                        out=cc_in[b][j].rearrange("(h d) w -> d h w", d=128),
                        in_=ctxT[:, :, j * TOK_SHARD : (j + 1) * TOK_SHARD],
                    )
                nc.gpsimd.collective_compute(
                    "AllToAll",
                    mybir.AluOpType.bypass,
                    replica_groups=[list(range(N_CORES))],
                    ins=[cc_in[b][:]],
                    outs=[cc_out[b][:]],
                )

            # ---------------- Phase A + attention(b0) interleaved ----------------
            with (
                tc.tile_pool(name="pb_pT", bufs=6) as ppT,
                tc.tile_pool(name="pb_cT", bufs=2) as pcT,
                tc.tile_pool(name="pb_rs_sb", bufs=2) as prs,
                tc.tile_pool(name="ps_mm", bufs=4, space="PSUM") as ps_mm,
                tc.tile_pool(name="ps_ctx", bufs=2, space="PSUM") as ps_ctx,
                tc.tile_pool(name="ps_rs", bufs=2, space="PSUM") as ps_rs,
            ):
                ctxT0 = pcT.tile([128, HPC, S], BF16, tag="ctxT", name="ctxT0")
                with (
                    tc.tile_pool(name="pa_w", bufs=1) as paw,
                    tc.tile_pool(name="pa_xt", bufs=3) as pxt,
                    tc.tile_pool(name="pa_xn", bufs=6) as pxn,
                    tc.tile_pool(name="pa_st", bufs=6) as pst,
                    tc.tile_pool(name="pa_A", bufs=2) as pA,
                    tc.tile_pool(name="pa_bc", bufs=3) as pbc,
                    tc.tile_pool(name="pa_ev", bufs=6) as pe,
                ):
                    w_sb = paw.tile([128, N_CC, 3 * HPC * HD], BF16)
                    for g in range(4):
                        nc.scalar.dma_start(
                            out=w_sb[:, g * 4 : (g + 1) * 4, :],
                            in_=wq[g * 512 : (g + 1) * 512, :].rearrange(
                                "(c p) f -> p c f", p=128
                            ),
                        )

                    def stats_sb(sb):
                        """bn_stats on natural x -> mu/istd/c1.  Each stat is
                        replicated across 128 columns (TS with the all-ones
                        tile), so one XBAR transpose of [128, 3*SB] yields the
                        partition-broadcast [128, SB] tiles directly:
                        bcast[:, s*4+tb, :] = stat_s(block tb) in every row."""
                        srep = pA.tile([128, 3 * SB], BF16, tag="A", name="srep")
                        for tb in range(4):
                            r0 = sb * SB + tb * 128
                            x_t = pxn.tile([128, HID], BF16, tag="xn", name="x_t")
                            nc.sync.dma_start(out=x_t[:], in_=xn[r0 : r0 + 128, :])
                            bn = pst.tile([128, 4, 6], F32, tag="bn", name="bn")
                            for c4 in range(4):
                                nc.vector.bn_stats(bn[:, c4, :], x_t[:, c4 * 512 : (c4 + 1) * 512])
                            mv = pst.tile([128, 2], F32, tag="mv", name="mv")
                            nc.vector.bn_aggr(mv[:], bn[:])
                            sd = pst.tile([128, 1], F32, tag="sd", name="sd")
                            nc.scalar.activation(
                                sd[:], mv[:, 1:2], mybir.ActivationFunctionType.Sqrt, bias=eps_t[:]
                            )
                            istd = pst.tile([128, 1], F32, tag="istd", name="istd")
                            nc.vector.reciprocal_approx_fast(out=istd[:], in_=sd[:])
                            blk = sb * 4 + tb
                            nc.vector.tensor_scalar(
                                out=sistd[:, blk : blk + 1], in0=istd[:],
                                scalar1=SCALE, scalar2=None, op0=MUL,
                            )
                            nc.vector.tensor_copy(istd_n[:, blk : blk + 1], istd[:])
                            nc.vector.tensor_scalar(
                                out=c1_n[:, blk : blk + 1], in0=mv[:, 0:1],
                                scalar1=istd[:], scalar2=None, op0=MUL,
                            )
                            cs = slice(0 * SB + tb * 128, 0 * SB + (tb + 1) * 128)
                            nc.vector.tensor_scalar(
                                out=srep[:, cs], in0=ones128[:],
                                scalar1=mv[:, 0:1], scalar2=None, op0=MUL,
                            )
                            cs = slice(1 * SB + tb * 128, 1 * SB + (tb + 1) * 128)
                            nc.vector.tensor_scalar(
                                out=srep[:, cs], in0=ones128[:],
                                scalar1=istd[:], scalar2=None, op0=MUL,
                            )
                            cs = slice(2 * SB + tb * 128, 2 * SB + (tb + 1) * 128)
                            nc.vector.tensor_scalar(
                                out=srep[:, cs], in0=ones128[:],
                                scalar1=c1_n[:, blk : blk + 1], scalar2=None, op0=MUL,
                            )
                        bc = pbc.tile([128, 12, 128], BF16, tag="bc", name="bc")
                        nc.scalar.dma_start_transpose(out=bc[:], in_=srep[:])
                        flat = bc[:].rearrange("p a b -> p (a b)")
                        mu_bc = flat[:, 0 * SB : 1 * SB]
                        istd_bc = flat[:, 1 * SB : 2 * SB]
                        c1_bc = flat[:, 2 * SB : 3 * SB]
                        return mu_bc, istd_bc, c1_bc

                    def load_xt(sb):
                        """Split into 4-cc groups so the first chain matmuls can
                        start as soon as the first 0.5MB lands."""
                        col0 = sb * SB
                        xt_t = pxt.tile([128, N_CC, SB], BF16, tag="xt", name="xt_t")
                        for g in range(4):
                            nc.sync.dma_start(
                                out=xt_t[:, g * 4 : (g + 1) * 4, :],
                                in_=xt[g * 512 : (g + 1) * 512, col0 : col0 + SB].rearrange(
                                    "(c p) t -> p c t", p=128
                                ),
                            )
                        return xt_t

                    def qkv_sb(sb, bcs, xt_t):
                        mu_bc, istd_bc, c1_bc = bcs
                        col0 = sb * SB
                        for h in range(HPC):
                            psq = ps_mm.tile([128, SB], F32, tag="mm", name="psq")
                            for cc in range(N_CC):
                                nc.tensor.matmul(
                                    psq[:],
                                    w_sb[:, cc, h * HD : (h + 1) * HD],
                                    xt_t[:, cc, :],
                                    start=(cc == 0),
                                    stop=(cc == N_CC - 1),
                                )
                            t1 = pe.tile([128, SB], BF16, tag="ev", name="t1")
                            nc.vector.tensor_mul(t1[:], psq[:], istd_bc)
                            # t2 = c1*rq - bq on ScalarE (bq negated on host)
                            t2 = pe.tile([128, SB], BF16, tag="ev", name="t2")
                            nc.scalar.activation(
                                t2[:], c1_bc, mybir.ActivationFunctionType.Identity,
                                scale=rqk_sb[:, h : h + 1], bias=bq_sb[:, h : h + 1],
                            )
                            nc.vector.tensor_sub(qT[:, h, col0 : col0 + SB], t1[:], t2[:])

                            psk = ps_mm.tile([128, SB], F32, tag="mm", name="psk")
                            for cc in range(N_CC):
                                nc.tensor.matmul(
                                    psk[:],
                                    w_sb[:, cc, HPC * HD + h * HD : HPC * HD + (h + 1) * HD],
                                    xt_t[:, cc, :],
                                    start=(cc == 0),
                                    stop=(cc == N_CC - 1),
                                )
                            # k = psk - mu*rk  (rk negated on host; istd_k folded
                            # into the exp scale)
                            nc.vector.scalar_tensor_tensor(
                                out=kT[:, h, col0 : col0 + SB],
                                in0=mu_bc, scalar=rqk_sb[:, 2 + h : 3 + h], in1=psk[:],
                                op0=MUL, op1=ADD,
                            )
                        for tb2 in range(4):
                            psv = ps_mm.tile([128, HPC * HD], F32, tag="mm", name="psv")
                            for cc in range(N_CC):
                                nc.tensor.matmul(
                                    psv[:],
                                    xt_t[:, cc, tb2 * 128 : (tb2 + 1) * 128],
                                    w_sb[:, cc, 2 * HPC * HD :],
                                    start=(cc == 0),
                                    stop=(cc == N_CC - 1),
                                )
                            blk = sb * 4 + tb2
                            va = pe.tile([128, HPC * HD], BF16, tag="ev", name="va")
                            nc.scalar.activation(
                                va[:], psv[:], mybir.ActivationFunctionType.Identity,
                                scale=istd_n[:, blk : blk + 1],
                            )
                            # v = va - c1*rv  (rv negated on host)
                            nc.vector.scalar_tensor_tensor(
                                out=v_sb[:, blk, :],
                                in0=rv_bc[:], scalar=c1_n[:, blk : blk + 1], in1=va[:],
                                op0=MUL, op1=ADD,
                            )

                    bcs = [None] * N_SB
                    xts = [None] * N_SB
                    bcs[0] = stats_sb(0)
                    xts[0] = load_xt(0)
                    bcs[1] = stats_sb(1)
                    xts[1] = load_xt(1)
                    qkv_sb(0, bcs[0], xts[0])
                    bcs[2] = stats_sb(2)
                    xts[2] = load_xt(2)
                    qkv_sb(1, bcs[1], xts[1])
                    bcs[3] = stats_sb(3)
                    xts[3] = load_xt(3)
                    qkv_sb(2, bcs[2], xts[2])
                    bcs[4] = stats_sb(4)
                    xts[4] = load_xt(4)
                    qkv_sb(3, bcs[3], xts[3])
                    bcs[5] = stats_sb(5)
                    xts[5] = load_xt(5)
                    qkv_sb(4, bcs[4], xts[4])
                    attn_qc(0, 0, ps_mm, ps_ctx, ps_rs, ppT, prs, ctxT0)
                    bcs[6] = stats_sb(6)
                    xts[6] = load_xt(6)
                    qkv_sb(5, bcs[5], xts[5])
                    attn_qc(0, 1, ps_mm, ps_ctx, ps_rs, ppT, prs, ctxT0)
                    bcs[7] = stats_sb(7)
                    xts[7] = load_xt(7)
                    qkv_sb(6, bcs[6], xts[6])
                    attn_qc(0, 2, ps_mm, ps_ctx, ps_rs, ppT, prs, ctxT0)
                    qkv_sb(7, bcs[7], xts[7])
                    attn_qc(0, 3, ps_mm, ps_ctx, ps_rs, ppT, prs, ctxT0)
                    ship_ctx(0, ctxT0)

                # ---------- Phase B: attention(b1) + output GEMMs ----------
                with (
                    tc.tile_pool(name="pb_ow", bufs=1) as pow_,
                    tc.tile_pool(name="pb_cf", bufs=2) as pcf,
                    tc.tile_pool(name="pb_o", bufs=2) as po,
                ):
                    ow_sb = pow_.tile([128, N_CC, HID], BF16)
                    for g in range(4):
                        nc.scalar.dma_start(
                            out=ow_sb[:, g * 4 : (g + 1) * 4, :],
                            in_=owt[g * 512 : (g + 1) * 512, :].rearrange(
                                "(c p) f -> p c f", p=128
                            ),
                        )
                    def load_cf(b):
                        cf = pcf.tile([128, N_CC, TOK_SHARD], BF16, tag="cf", name=f"cf{b}")
                        src = cc_out[b].rearrange("j (h d) w -> d (j h) w", d=128)
                        nc.sync.dma_start(out=cf[:, 0:8, :], in_=src[:, 0:8, :])
                        nc.sync.dma_start(out=cf[:, 8:16, :], in_=src[:, 8:16, :])
                        return cf

                    cf0 = load_cf(0)

                    def outg_tb(b, cf, tb):
                        o_t = po.tile([128, HID], F32, tag="o", name="o_t")
                        for half in range(2):
                            pso = [
                                ps_mm.tile([128, 512], F32, tag="mm", name=f"pso{nb}")
                                for nb in (2 * half, 2 * half + 1)
                            ]
                            for cc in range(N_CC):
                                for i, nb in enumerate((2 * half, 2 * half + 1)):
                                    nc.tensor.matmul(
                                        pso[i][:],
                                        cf[:, cc, tb * 128 : (tb + 1) * 128],
                                        ow_sb[:, cc, nb * 512 : (nb + 1) * 512],
                                        start=(cc == 0),
                                        stop=(cc == N_CC - 1),
                                    )
                            for i, nb in enumerate((2 * half, 2 * half + 1)):
                                nc.vector.tensor_add(
                                    o_t[:, nb * 512 : (nb + 1) * 512], pso[i][:],
                                    ob_bc[:, nb * 512 : (nb + 1) * 512],
                                )
                        nc.sync.dma_start(
                            out=out[b * TOK_SHARD + tb * 128 : b * TOK_SHARD + (tb + 1) * 128, :],
                            in_=o_t[:],
                        )

                    ctxT1 = pcT.tile([128, HPC, S], BF16, tag="ctxT", name="ctxT1")
                    attn_qc(1, 0, ps_mm, ps_ctx, ps_rs, ppT, prs, ctxT1)
                    attn_qc(1, 1, ps_mm, ps_ctx, ps_rs, ppT, prs, ctxT1)
                    attn_qc(1, 2, ps_mm, ps_ctx, ps_rs, ppT, prs, ctxT1)
                    attn_qc(1, 3, ps_mm, ps_ctx, ps_rs, ppT, prs, ctxT1)
                    ship_ctx(1, ctxT1)
                    # outG(b0) fills the PE while A2A(b1) is in flight
                    outg_tb(0, cf0, 0)
                    cf1 = load_cf(1)
                    outg_tb(0, cf0, 1)
                    outg_tb(1, cf1, 0)
                    outg_tb(1, cf1, 1)

    nc.compile()
    return nc


_CACHE = {}


def _get_nc(apply_mask: bool):
    if apply_mask not in _CACHE:
        _CACHE[apply_mask] = _build(apply_mask)
    return _CACHE[apply_mask]


def _prep_in_maps(input, input_mask, norm_w, norm_b, attn_qkvw, attn_qkvb, attn_ow):
    bf16 = ml_dtypes.bfloat16
    x = np.asarray(input, dtype=np.float32).reshape(T, HID)
    w = np.asarray(attn_qkvw, dtype=np.float32)
    nw = np.asarray(norm_w, dtype=np.float32)
    nb = np.asarray(norm_b, dtype=np.float32)
    qb_ = np.asarray(attn_qkvb, dtype=np.float32)
    ow_f = np.asarray(attn_ow, dtype=np.float32)
    ow = np.ascontiguousarray(ow_f.astype(bf16))
    mask = np.asarray(input_mask, dtype=np.float32).reshape(B, S)

    xt = np.ascontiguousarray(x.T.astype(bf16))  # [HID, T]
    xn = np.ascontiguousarray(x.astype(bf16))  # [T, HID]

    w_eff = nw[:, None] * w  # fold LN gamma into QKV weight
    b_eff = nb @ w + qb_  # fold LN beta into QKV bias
    colsum = w_eff.sum(axis=0)  # [3*HID]
    obias_full = np.ascontiguousarray(
        (b_eff[2 * HID :] @ ow_f).reshape(1, HID).astype(bf16)
    )

    apply_mask = bool(np.any(mask != 0.0))
    if apply_mask:
        # per-key layout: [128 partitions (k within block), B * 16 key-blocks]
        mprep = np.ascontiguousarray(
            mask.reshape(B, S // 128, 128).transpose(2, 0, 1).reshape(128, B * (S // 128))
        )
    in_maps = []
    for i in range(N_CORES):
        cols = []
        for part in range(3):  # q, k, v column shards for this core's heads
            c0 = part * HID + i * HPC * HD
            cols.append(w_eff[:, c0 : c0 + HPC * HD])
        wqkv_i = np.ascontiguousarray(np.concatenate(cols, axis=1).astype(bf16))

        q0 = i * HPC * HD
        k0 = HID + i * HPC * HD
        v0 = 2 * HID + i * HPC * HD
        rqk_i = np.ascontiguousarray(
            np.stack(
                [
                    colsum[q0 : q0 + HD],
                    colsum[q0 + HD : q0 + 2 * HD],
                    -colsum[k0 : k0 + HD],
                    -colsum[k0 + HD : k0 + 2 * HD],
                ],
                axis=1,
            ).astype(np.float32)
        )
        # negated: the ScalarE Identity epilogue computes c1*rq + (-bq)
        bq_i = np.ascontiguousarray(
            np.stack(
                [-b_eff[q0 : q0 + HD], -b_eff[q0 + HD : q0 + 2 * HD]], axis=1
            ).astype(np.float32)
        )
        rv_i = np.ascontiguousarray(
            (-colsum[v0 : v0 + HPC * HD]).reshape(1, HPC * HD).astype(bf16)
        )
        m = {
            "xt": xt,
            "xn": xn,
            "qkvw": wqkv_i,
            "rqk": rqk_i,
            "bq": bq_i,
            "rv": rv_i,
            "obias": obias_full,
            "ow": ow,
        }
        if apply_mask:
            m["imask"] = mprep
        in_maps.append(m)
    return in_maps, apply_mask


def _run(inputs: dict, trace: bool = False):
    from concourse.bass_utils import run_bass_kernel_spmd

    in_maps, apply_mask = _prep_in_maps(**inputs)
    nc = _get_nc(apply_mask)
    res = run_bass_kernel_spmd(nc, in_maps, list(range(N_CORES)), trace=trace)
    out = np.empty((B, S, HID), dtype=np.float32)
    for j in range(N_CORES):
        o = res.results[j]["out"]
        for b in range(B):
            out[b, j * TOK_SHARD : (j + 1) * TOK_SHARD] = o[b * TOK_SHARD : (b + 1) * TOK_SHARD]
    return out, res


def kernel(**inputs) -> np.ndarray:
    out, _ = _run(inputs, trace=False)
    return out


# revision 23
# speedup vs baseline: 1.2192x; 1.0003x over previous
"""Tensor-parallel DeepSpeed-style self-attention block on 8 TRN2 NeuronCores.

v4 strategy (fold LN into GEMM epilogues; kill startup + queue serialization):
  - Host pre-transposes the input to xT (bf16) and also passes a natural
    bf16 copy for LayerNorm statistics.  The QKV GEMM runs directly on the
    RAW xT -- no z pass, no on-device zT transposes -- so the first matmul
    fires as soon as the first weight/xT chunks land (~10us vs ~67us).
  - LN is applied algebraically in the PSUM evacuation:
        q = istd .* psq - (mu*istd) .* rq + bq         (DVE, 3 ops)
        k = psk - mu .* rk                             (DVE STT, 1 op;
          the istd_k factor rides for free in the exp's per-partition scale)
        v = istd .* (psv - mu .* rv)                   (DVE, 2 ops;
          the V bias is exact via softmax-rows-sum-to-1 and is folded into a
          host-precomputed output bias added in the output-GEMM epilogue)
    where rq/rk/rv are host-precomputed column sums of the LN-folded QKV
    weight and mu/istd come from on-device bn_stats over the natural copy,
    transposed via one tiny [128,128] XBAR per superblock and broadcast with
    GpSimd partition_broadcast.
  - Attention per (batch, head) fully transposed as v3, but the softmax
    rowsum matmul uses an all-ones [128,128] stationary (M=128, not M=1):
    the PSUM result is the rowsum pre-broadcast to all partitions, so the
    normalize is just reciprocal_approx_fast + one tensor_tensor multiply.
    No ScalarE copies and -- critically -- NO GpSimd broadcasts in
    attention, so the AllToAll collectives are not stuck behind attention
    work in the GpSimd queue: A2A(b0) now fires at the start of phase B and
    hides under attention(b1).
  - Output GEMM runs cc-outer over nb-pairs so a stationary cf chunk is
    reused across two 512-wide streams; outG(b0) interleaves into the tail
    of attention(b1) and covers A2A(b1).
"""

import sys

if "/opt/trn_rl_repo" not in sys.path:
    sys.path.insert(0, "/opt/trn_rl_repo")

# --- shim antenv.axon_hooks (missing in this image) so trace=True can NTFF-profile ---
import types, ctypes, contextlib


def _make_ntff_hook(so_path="/opt/axon/libaxon_pjrt.so"):
    try:
        lib = ctypes.CDLL(so_path)
    except OSError:
        return None
    if not hasattr(lib, "axon_start_nrt_profile"):
        return None
    lib.axon_start_nrt_profile.argtypes = [ctypes.POINTER(ctypes.c_int64), ctypes.c_size_t]
    lib.axon_start_nrt_profile.restype = ctypes.c_int64
    lib.axon_stop_nrt_profile.argtypes = [ctypes.c_char_p]
    lib.axon_stop_nrt_profile.restype = ctypes.c_int64

    @contextlib.contextmanager
    def _hook(output_dir, device_ids):
        import jax

        jax.devices()
        if device_ids:
            ids = (ctypes.c_int64 * len(device_ids))(*device_ids)
            rc = lib.axon_start_nrt_profile(ids, len(device_ids))
        else:
            rc = lib.axon_start_nrt_profile(None, 0)
        if rc != 0:
            raise RuntimeError(f"axon_start_nrt_profile rc={rc}")
        try:
            yield
        finally:
            n = lib.axon_stop_nrt_profile(str(output_dir).encode())
            if n < 0:
                raise RuntimeError(f"axon_stop_nrt_profile rc={n}")

    return _hook


if "antenv.axon_hooks" not in sys.modules:
    _m = types.ModuleType("antenv.axon_hooks")
    _m.get_axon_ntff_profile_hook = lambda: _make_ntff_hook()
    sys.modules["antenv.axon_hooks"] = _m
# --- end shim ---

import numpy as np
import ml_dtypes  # noqa: F401  (bf16 numpy dtype registration)

from concourse import bacc, tile, mybir
from concourse.masks import make_upper_triangular

B, S, HID = 2, 2048, 2048
HEADS = 16
HD = 128
T = B * S
N_CORES = 8
HPC = HEADS // N_CORES  # 2 heads per core
EPS = 1e-6
SCALE = 1.0 / float(np.sqrt(HD))

F32 = mybir.dt.float32
BF16 = mybir.dt.bfloat16

SB = 512  # tokens per LN/QKV superblock
N_SB = T // SB  # 8
N_CC = HID // 128  # 16 contraction chunks
TOK_SHARD = S // N_CORES  # 256 tokens per (batch, core) after A2A
QC = 512  # attention q-chunk width
MUL = mybir.AluOpType.mult
SUB = mybir.AluOpType.subtract
ADD = mybir.AluOpType.add


def _build(apply_mask: bool):
    nc = bacc.Bacc("TRN2", target_bir_lowering=False, debug=False, num_devices=N_CORES)

    xt = nc.dram_tensor("xt", [HID, T], BF16, kind="ExternalInput").ap()
    xn = nc.dram_tensor("xn", [T, HID], BF16, kind="ExternalInput").ap()
    wq = nc.dram_tensor("qkvw", [HID, 3 * HPC * HD], BF16, kind="ExternalInput").ap()
    rqk = nc.dram_tensor("rqk", [128, 4], F32, kind="ExternalInput").ap()
    bqd = nc.dram_tensor("bq", [128, HPC], F32, kind="ExternalInput").ap()
    rvd = nc.dram_tensor("rv", [1, HPC * HD], BF16, kind="ExternalInput").ap()
    obd = nc.dram_tensor("obias", [1, HID], BF16, kind="ExternalInput").ap()
    owt = nc.dram_tensor("ow", [HID, HID], BF16, kind="ExternalInput").ap()
    out = nc.dram_tensor("out", [B * TOK_SHARD, HID], F32, kind="ExternalOutput").ap()
    if apply_mask:
        imask = nc.dram_tensor("imask", [128, B * (S // 128)], F32, kind="ExternalInput").ap()

    cc_in = [nc.dram_tensor(f"cc_in{b}", [N_CORES, HPC * HD, TOK_SHARD], BF16).ap() for b in range(B)]
    cc_out = [nc.dram_tensor(f"cc_out{b}", [N_CORES, HPC * HD, TOK_SHARD], BF16).ap() for b in range(B)]

    with tile.TileContext(nc) as tc:
        with tc.tile_pool(name="persist", bufs=1) as pers:
            ones128 = pers.tile([128, 128], BF16)
            nc.gpsimd.memset(ones128[:], 1.0)
            eps_t = pers.tile([128, 1], F32)
            nc.gpsimd.memset(eps_t[:], EPS)
            trif = pers.tile([128, 128], F32)
            make_upper_triangular(nc, trif[:], val=1.0, diag=True)
            tri01 = pers.tile([128, 128], BF16)
            nc.vector.tensor_copy(tri01[:], trif[:])
            rqk_sb = pers.tile([128, 4], F32)
            nc.scalar.dma_start(out=rqk_sb[:], in_=rqk[:])
            bq_sb = pers.tile([128, HPC], F32)
            nc.scalar.dma_start(out=bq_sb[:], in_=bqd[:])
            rv_row = pers.tile([1, HPC * HD], BF16)
            nc.scalar.dma_start(out=rv_row[:], in_=rvd[:])
            rv_bc = pers.tile([128, HPC * HD], BF16)
            nc.gpsimd.partition_broadcast(rv_bc[:], rv_row[:])
            ob_row = pers.tile([1, HID], BF16)
            nc.scalar.dma_start(out=ob_row[:], in_=obd[:])
            ob_bc = pers.tile([128, HID], BF16)
            nc.gpsimd.partition_broadcast(ob_bc[:], ob_row[:])
            if apply_mask:
                msk = pers.tile([128, B * (S // 128)], F32)
                nc.scalar.dma_start(out=msk[:], in_=imask[:])

            qT = pers.tile([128, HPC, T], BF16)  # [d, head, tok]
            kT = pers.tile([128, HPC, T], BF16)
            v_sb = pers.tile([128, T // 128, HPC * HD], BF16)  # [tok128, blk, hcol]
            sistd = pers.tile([128, T // 128], F32)  # SCALE * istd per token-block
            istd_n = pers.tile([128, T // 128], F32)  # 1/sd, natural layout
            c1_n = pers.tile([128, T // 128], F32)  # mu/sd, natural layout

            # ---------- attention emitter (transposed, sw-pipelined) ----------
            def attn_qc(b, qc, ps_mm, ps_ctx, ps_rs, ppT, prs, ctxT):
                nkb = 4 * qc + 4
                ctx_ps = [ps_ctx.tile([128, QC], F32, tag="ctx", name=f"ctx{h}") for h in range(HPC)]
                rs_ps = [ps_rs.tile([128, QC], F32, tag="rs", name=f"rs{h}") for h in range(HPC)]
                kbs = list(range(nkb - 1, -1, -1))
                pend = {}

                def emit_sc(kb):
                    c0 = max(0, (kb - 4 * qc) * 128)
                    w = QC - c0
                    for h in range(HPC):
                        sc = ps_mm.tile([128, QC], F32, tag="mm", name="sc")
                        nc.tensor.matmul(
                            sc[:, :w],
                            kT[:, h, b * S + kb * 128 : b * S + kb * 128 + 128],
                            qT[:, h, b * S + qc * QC + c0 : b * S + qc * QC + c0 + w],
                            start=True,
                            stop=True,
                        )
                        pt = ppT.tile([128, QC], BF16, tag="pt", name="pt")
                        bias = msk[:, b * 16 + kb : b * 16 + kb + 1] if apply_mask else 0.0
                        nc.scalar.activation(
                            pt[:, :w], sc[:, :w], mybir.ActivationFunctionType.Exp,
                            scale=sistd[:, b * 16 + kb : b * 16 + kb + 1], bias=bias,
                        )
                        if kb >= 4 * qc:  # causal diagonal block
                            nc.vector.tensor_mul(pt[:, 0:128], pt[:, 0:128], tri01[:])
                        pend[(h, kb)] = (pt, c0, w)

                def emit_consume(kb):
                    for h in range(HPC):
                        pt, c0, w = pend.pop((h, kb))
                        nc.tensor.matmul(
                            rs_ps[h][:, c0:QC], ones128[:], pt[:, :w],
                            start=(kb == kbs[0]), stop=(kb == 0),
                        )
                        nc.tensor.matmul(
                            ctx_ps[h][:, c0:QC],
                            v_sb[:, b * 16 + kb, h * HD : (h + 1) * HD],
                            pt[:, :w],
                            start=(kb == kbs[0]), stop=(kb == 0),
                        )

                for i, kb in enumerate(kbs):
                    emit_sc(kb)
                    if i > 0:
                        emit_consume(kbs[i - 1])
                emit_consume(kbs[-1])

                for h in range(HPC):
                    rsi = prs.tile([128, QC], F32, tag="rsi", name="rsi")
                    nc.vector.reciprocal_approx_fast(out=rsi[:], in_=rs_ps[h][:])
                    nc.vector.tensor_mul(
                        ctxT[:, h, qc * QC : (qc + 1) * QC], ctx_ps[h][:], rsi[:]
                    )

            def ship_ctx(b, ctxT):
                for j in range(N_CORES):
                    nc.sync.dma_start(
                        out=cc_in[b][j].rearrange("(h d) w -> d h w", d=128),
                        in_=ctxT[:, :, j * TOK_SHARD : (j + 1) * TOK_SHARD],
                    )
                nc.gpsimd.collective_compute(
                    "AllToAll",
                    mybir.AluOpType.bypass,
                    replica_groups=[list(range(N_CORES))],
                    ins=[cc_in[b][:]],
                    outs=[cc_out[b][:]],
                )

            # ---------------- Phase A + attention(b0) interleaved ----------------
            with (
                tc.tile_pool(name="pb_pT", bufs=6) as ppT,
                tc.tile_pool(name="pb_cT", bufs=2) as pcT,
                tc.tile_pool(name="pb_rs_sb", bufs=2) as prs,
                tc.tile_pool(name="ps_mm", bufs=4, space="PSUM") as ps_mm,
                tc.tile_pool(name="ps_ctx", bufs=2, space="PSUM") as ps_ctx,
                tc.tile_pool(name="ps_rs", bufs=2, space="PSUM") as ps_rs,
            ):
                ctxT0 = pcT.tile([128, HPC, S], BF16, tag="ctxT", name="ctxT0")
                with (
                    tc.tile_pool(name="pa_w", bufs=1) as paw,
                    tc.tile_pool(name="pa_xt", bufs=3) as pxt,
                    tc.tile_pool(name="pa_xn", bufs=6) as pxn,
                    tc.tile_pool(name="pa_st", bufs=6) as pst,
                    tc.tile_pool(name="pa_A", bufs=2) as pA,
                    tc.tile_pool(name="pa_bc", bufs=3) as pbc,
                    tc.tile_pool(name="pa_ev", bufs=6) as pe,
                ):
                    w_sb = paw.tile([128, N_CC, 3 * HPC * HD], BF16)
                    for g in range(4):
                        nc.scalar.dma_start(
                            out=w_sb[:, g * 4 : (g + 1) * 4, :],
                            in_=wq[g * 512 : (g + 1) * 512, :].rearrange(
                                "(c p) f -> p c f", p=128
                            ),
                        )

                    def stats_sb(sb):
                        """bn_stats on natural x -> mu/istd/c1.  Each stat is
                        replicated across 128 columns (TS with the all-ones
                        tile), so one XBAR transpose of [128, 3*SB] yields the
                        partition-broadcast [128, SB] tiles directly:
                        bcast[:, s*4+tb, :] = stat_s(block tb) in every row."""
                        srep = pA.tile([128, 3 * SB], BF16, tag="A", name="srep")
                        for tb in range(4):
                            r0 = sb * SB + tb * 128
                            x_t = pxn.tile([128, HID], BF16, tag="xn", name="x_t")
                            nc.scalar.dma_start(out=x_t[:], in_=xn[r0 : r0 + 128, :])
                            bn = pst.tile([128, 4, 6], F32, tag="bn", name="bn")
                            for c4 in range(4):
                                nc.vector.bn_stats(bn[:, c4, :], x_t[:, c4 * 512 : (c4 + 1) * 512])
                            mv = pst.tile([128, 2], F32, tag="mv", name="mv")
                            nc.vector.bn_aggr(mv[:], bn[:])
                            sd = pst.tile([128, 1], F32, tag="sd", name="sd")
                            nc.scalar.activation(
                                sd[:], mv[:, 1:2], mybir.ActivationFunctionType.Sqrt, bias=eps_t[:]
                            )
                            istd = pst.tile([128, 1], F32, tag="istd", name="istd")
                            nc.vector.reciprocal_approx_fast(out=istd[:], in_=sd[:])
                            blk = sb * 4 + tb
                            nc.vector.tensor_scalar(
                                out=sistd[:, blk : blk + 1], in0=istd[:],
                                scalar1=SCALE, scalar2=None, op0=MUL,
                            )
                            nc.vector.tensor_copy(istd_n[:, blk : blk + 1], istd[:])
                            nc.vector.tensor_scalar(
                                out=c1_n[:, blk : blk + 1], in0=mv[:, 0:1],
                                scalar1=istd[:], scalar2=None, op0=MUL,
                            )
                            cs = slice(0 * SB + tb * 128, 0 * SB + (tb + 1) * 128)
                            nc.vector.tensor_scalar(
                                out=srep[:, cs], in0=ones128[:],
                                scalar1=mv[:, 0:1], scalar2=None, op0=MUL,
                            )
                            cs = slice(1 * SB + tb * 128, 1 * SB + (tb + 1) * 128)
                            nc.vector.tensor_scalar(
                                out=srep[:, cs], in0=ones128[:],
                                scalar1=istd[:], scalar2=None, op0=MUL,
                            )
                            cs = slice(2 * SB + tb * 128, 2 * SB + (tb + 1) * 128)
                            nc.vector.tensor_scalar(
                                out=srep[:, cs], in0=ones128[:],
                                scalar1=c1_n[:, blk : blk + 1], scalar2=None, op0=MUL,
                            )
                        bc = pbc.tile([128, 12, 128], BF16, tag="bc", name="bc")
                        nc.scalar.dma_start_transpose(out=bc[:], in_=srep[:])
                        flat = bc[:].rearrange("p a b -> p (a b)")
                        mu_bc = flat[:, 0 * SB : 1 * SB]
                        istd_bc = flat[:, 1 * SB : 2 * SB]
                        c1_bc = flat[:, 2 * SB : 3 * SB]
                        return mu_bc, istd_bc, c1_bc

                    def load_xt(sb):
                        """Split into 4-cc groups so the first chain matmuls can
                        start as soon as the first 0.5MB lands."""
                        col0 = sb * SB
                        xt_t = pxt.tile([128, N_CC, SB], BF16, tag="xt", name="xt_t")
                        for g in range(4):
                            nc.sync.dma_start(
                                out=xt_t[:, g * 4 : (g + 1) * 4, :],
                                in_=xt[g * 512 : (g + 1) * 512, col0 : col0 + SB].rearrange(
                                    "(c p) t -> p c t", p=128
                                ),
                            )
                        return xt_t

                    def qkv_sb(sb, bcs, xt_t):
                        mu_bc, istd_bc, c1_bc = bcs
                        col0 = sb * SB
                        for h in range(HPC):
                            psq = ps_mm.tile([128, SB], F32, tag="mm", name="psq")
                            for cc in range(N_CC):
                                nc.tensor.matmul(
                                    psq[:],
                                    w_sb[:, cc, h * HD : (h + 1) * HD],
                                    xt_t[:, cc, :],
                                    start=(cc == 0),
                                    stop=(cc == N_CC - 1),
                                )
                            t1 = pe.tile([128, SB], BF16, tag="ev", name="t1")
                            nc.vector.tensor_mul(t1[:], psq[:], istd_bc)
                            # t2 = c1*rq - bq on ScalarE (bq negated on host)
                            t2 = pe.tile([128, SB], BF16, tag="ev", name="t2")
                            nc.scalar.activation(
                                t2[:], c1_bc, mybir.ActivationFunctionType.Identity,
                                scale=rqk_sb[:, h : h + 1], bias=bq_sb[:, h : h + 1],
                            )
                            nc.vector.tensor_sub(qT[:, h, col0 : col0 + SB], t1[:], t2[:])

                            psk = ps_mm.tile([128, SB], F32, tag="mm", name="psk")
                            for cc in range(N_CC):
                                nc.tensor.matmul(
                                    psk[:],
                                    w_sb[:, cc, HPC * HD + h * HD : HPC * HD + (h + 1) * HD],
                                    xt_t[:, cc, :],
                                    start=(cc == 0),
                                    stop=(cc == N_CC - 1),
                                )
                            # k = psk - mu*rk  (rk negated on host; istd_k folded
                            # into the exp scale)
                            nc.vector.scalar_tensor_tensor(
                                out=kT[:, h, col0 : col0 + SB],
                                in0=mu_bc, scalar=rqk_sb[:, 2 + h : 3 + h], in1=psk[:],
                                op0=MUL, op1=ADD,
                            )
                        for tb2 in range(4):
                            psv = ps_mm.tile([128, HPC * HD], F32, tag="mm", name="psv")
                            for cc in range(N_CC):
                                nc.tensor.matmul(
                                    psv[:],
                                    xt_t[:, cc, tb2 * 128 : (tb2 + 1) * 128],
                                    w_sb[:, cc, 2 * HPC * HD :],
                                    start=(cc == 0),
                                    stop=(cc == N_CC - 1),
                                )
                            blk = sb * 4 + tb2
                            va = pe.tile([128, HPC * HD], BF16, tag="ev", name="va")
                            nc.vector.tensor_scalar(
                                out=va[:], in0=psv[:],
                                scalar1=istd_n[:, blk : blk + 1], scalar2=None, op0=MUL,
                            )
                            # v = va - c1*rv  (rv negated on host)
                            nc.vector.scalar_tensor_tensor(
                                out=v_sb[:, blk, :],
                                in0=rv_bc[:], scalar=c1_n[:, blk : blk + 1], in1=va[:],
                                op0=MUL, op1=ADD,
                            )

                    bcs = [None] * N_SB
                    xts = [None] * N_SB
                    bcs[0] = stats_sb(0)
                    xts[0] = load_xt(0)
                    bcs[1] = stats_sb(1)
                    xts[1] = load_xt(1)
                    qkv_sb(0, bcs[0], xts[0])
                    bcs[2] = stats_sb(2)
                    xts[2] = load_xt(2)
                    qkv_sb(1, bcs[1], xts[1])
                    bcs[3] = stats_sb(3)
                    xts[3] = load_xt(3)
                    qkv_sb(2, bcs[2], xts[2])
                    bcs[4] = stats_sb(4)
                    xts[4] = load_xt(4)
                    qkv_sb(3, bcs[3], xts[3])
                    bcs[5] = stats_sb(5)
                    xts[5] = load_xt(5)
                    qkv_sb(4, bcs[4], xts[4])
                    attn_qc(0, 0, ps_mm, ps_ctx, ps_rs, ppT, prs, ctxT0)
                    bcs[6] = stats_sb(6)
                    xts[6] = load_xt(6)
                    qkv_sb(5, bcs[5], xts[5])
                    attn_qc(0, 1, ps_mm, ps_ctx, ps_rs, ppT, prs, ctxT0)
                    bcs[7] = stats_sb(7)
                    xts[7] = load_xt(7)
                    qkv_sb(6, bcs[6], xts[6])
                    attn_qc(0, 2, ps_mm, ps_ctx, ps_rs, ppT, prs, ctxT0)
                    qkv_sb(7, bcs[7], xts[7])
                    attn_qc(0, 3, ps_mm, ps_ctx, ps_rs, ppT, prs, ctxT0)
                    ship_ctx(0, ctxT0)

                # ---------- Phase B: attention(b1) + output GEMMs ----------
                with (
                    tc.tile_pool(name="pb_ow", bufs=1) as pow_,
                    tc.tile_pool(name="pb_cf", bufs=2) as pcf,
                    tc.tile_pool(name="pb_o", bufs=2) as po,
                ):
                    ow_sb = pow_.tile([128, N_CC, HID], BF16)
                    for g in range(4):
                        nc.scalar.dma_start(
                            out=ow_sb[:, g * 4 : (g + 1) * 4, :],
                            in_=owt[g * 512 : (g + 1) * 512, :].rearrange(
                                "(c p) f -> p c f", p=128
                            ),
                        )
                    def load_cf(b):
                        cf = pcf.tile([128, N_CC, TOK_SHARD], BF16, tag="cf", name=f"cf{b}")
                        src = cc_out[b].rearrange("j (h d) w -> d (j h) w", d=128)
                        nc.sync.dma_start(out=cf[:, 0:8, :], in_=src[:, 0:8, :])
                        nc.sync.dma_start(out=cf[:, 8:16, :], in_=src[:, 8:16, :])
                        return cf

                    cf0 = load_cf(0)

                    def outg_tb(b, cf, tb):
                        o_t = po.tile([128, HID], F32, tag="o", name="o_t")
                        for half in range(2):
                            pso = [
                                ps_mm.tile([128, 512], F32, tag="mm", name=f"pso{nb}")
                                for nb in (2 * half, 2 * half + 1)
                            ]
                            for cc in range(N_CC):
                                for i, nb in enumerate((2 * half, 2 * half + 1)):
                                    nc.tensor.matmul(
                                        pso[i][:],
                                        cf[:, cc, tb * 128 : (tb + 1) * 128],
                                        ow_sb[:, cc, nb * 512 : (nb + 1) * 512],
                                        start=(cc == 0),
                                        stop=(cc == N_CC - 1),
                                    )
                            for i, nb in enumerate((2 * half, 2 * half + 1)):
                                nc.vector.tensor_add(
                                    o_t[:, nb * 512 : (nb + 1) * 512], pso[i][:],
                                    ob_bc[:, nb * 512 : (nb + 1) * 512],
                                )
                        nc.sync.dma_start(
                            out=out[b * TOK_SHARD + tb * 128 : b * TOK_SHARD + (tb + 1) * 128, :],
                            in_=o_t[:],
                        )

                    ctxT1 = pcT.tile([128, HPC, S], BF16, tag="ctxT", name="ctxT1")
                    attn_qc(1, 0, ps_mm, ps_ctx, ps_rs, ppT, prs, ctxT1)
                    attn_qc(1, 1, ps_mm, ps_ctx, ps_rs, ppT, prs, ctxT1)
                    attn_qc(1, 2, ps_mm, ps_ctx, ps_rs, ppT, prs, ctxT1)
                    attn_qc(1, 3, ps_mm, ps_ctx, ps_rs, ppT, prs, ctxT1)
                    ship_ctx(1, ctxT1)
                    # outG(b0) fills the PE while A2A(b1) is in flight
                    outg_tb(0, cf0, 0)
                    cf1 = load_cf(1)
                    outg_tb(0, cf0, 1)
                    outg_tb(1, cf1, 0)
                    outg_tb(1, cf1, 1)

    nc.compile()
    return nc


_CACHE = {}


def _get_nc(apply_mask: bool):
    if apply_mask not in _CACHE:
        _CACHE[apply_mask] = _build(apply_mask)
    return _CACHE[apply_mask]


def _prep_in_maps(input, input_mask, norm_w, norm_b, attn_qkvw, attn_qkvb, attn_ow):
    bf16 = ml_dtypes.bfloat16
    x = np.asarray(input, dtype=np.float32).reshape(T, HID)
    w = np.asarray(attn_qkvw, dtype=np.float32)
    nw = np.asarray(norm_w, dtype=np.float32)
    nb = np.asarray(norm_b, dtype=np.float32)
    qb_ = np.asarray(attn_qkvb, dtype=np.float32)
    ow_f = np.asarray(attn_ow, dtype=np.float32)
    ow = np.ascontiguousarray(ow_f.astype(bf16))
    mask = np.asarray(input_mask, dtype=np.float32).reshape(B, S)

    xt = np.ascontiguousarray(x.T.astype(bf16))  # [HID, T]
    xn = np.ascontiguousarray(x.astype(bf16))  # [T, HID]

    w_eff = nw[:, None] * w  # fold LN gamma into QKV weight
    b_eff = nb @ w + qb_  # fold LN beta into QKV bias
    colsum = w_eff.sum(axis=0)  # [3*HID]
    obias_full = np.ascontiguousarray(
        (b_eff[2 * HID :] @ ow_f).reshape(1, HID).astype(bf16)
    )

    apply_mask = bool(np.any(mask != 0.0))
    if apply_mask:
        # per-key layout: [128 partitions (k within block), B * 16 key-blocks]
        mprep = np.ascontiguousarray(
            mask.reshape(B, S // 128, 128).transpose(2, 0, 1).reshape(128, B * (S // 128))
        )
    in_maps = []
    for i in range(N_CORES):
        cols = []
        for part in range(3):  # q, k, v column shards for this core's heads
            c0 = part * HID + i * HPC * HD
            cols.append(w_eff[:, c0 : c0 + HPC * HD])
        wqkv_i = np.ascontiguousarray(np.concatenate(cols, axis=1).astype(bf16))

        q0 = i * HPC * HD
        k0 = HID + i * HPC * HD
        v0 = 2 * HID + i * HPC * HD
        rqk_i = np.ascontiguousarray(
            np.stack(
                [
                    colsum[q0 : q0 + HD],
                    colsum[q0 + HD : q0 + 2 * HD],
                    -colsum[k0 : k0 + HD],
                    -colsum[k0 + HD : k0 + 2 * HD],
                ],
                axis=1,
            ).astype(np.float32)
        )
        # negated: the ScalarE Identity epilogue computes c1*rq + (-bq)
        bq_i = np.ascontiguousarray(
            np.stack(
                [-b_eff[q0 : q0 + HD], -b_eff[q0 + HD : q0 + 2 * HD]], axis=1
            ).astype(np.float32)
        )
        rv_i = np.ascontiguousarray(
            (-colsum[v0 : v0 + HPC * HD]).reshape(1, HPC * HD).astype(bf16)
        )
        m = {
            "xt": xt,
            "xn": xn,
            "qkvw": wqkv_i,
            "rqk": rqk_i,
            "bq": bq_i,
            "rv": rv_i,
            "obias": obias_full,
            "ow": ow,
        }
        if apply_mask:
            m["imask"] = mprep
        in_maps.append(m)
    return in_maps, apply_mask


def _run(inputs: dict, trace: bool = False):
    from concourse.bass_utils import run_bass_kernel_spmd

    in_maps, apply_mask = _prep_in_maps(**inputs)
    nc = _get_nc(apply_mask)
    res = run_bass_kernel_spmd(nc, in_maps, list(range(N_CORES)), trace=trace)
    out = np.empty((B, S, HID), dtype=np.float32)
    for j in range(N_CORES):
        o = res.results[j]["out"]
        for b in range(B):
            out[b, j * TOK_SHARD : (j + 1) * TOK_SHARD] = o[b * TOK_SHARD : (b + 1) * TOK_SHARD]
    return out, res


def kernel(**inputs) -> np.ndarray:
    out, _ = _run(inputs, trace=False)
    return out


# revision 25
# speedup vs baseline: 1.2427x; 1.0192x over previous
"""Tensor-parallel DeepSpeed-style self-attention block on 8 TRN2 NeuronCores.

v4 strategy (fold LN into GEMM epilogues; kill startup + queue serialization):
  - Host pre-transposes the input to xT (bf16) and also passes a natural
    bf16 copy for LayerNorm statistics.  The QKV GEMM runs directly on the
    RAW xT -- no z pass, no on-device zT transposes -- so the first matmul
    fires as soon as the first weight/xT chunks land (~10us vs ~67us).
  - LN is applied algebraically in the PSUM evacuation:
        q = istd .* psq - (mu*istd) .* rq + bq         (DVE, 3 ops)
        k = psk - mu .* rk                             (DVE STT, 1 op;
          the istd_k factor rides for free in the exp's per-partition scale)
        v = istd .* (psv - mu .* rv)                   (DVE, 2 ops;
          the V bias is exact via softmax-rows-sum-to-1 and is folded into a
          host-precomputed output bias added in the output-GEMM epilogue)
    where rq/rk/rv are host-precomputed column sums of the LN-folded QKV
    weight and mu/istd come from on-device bn_stats over the natural copy.
    Each stat is replicated across 128 columns (tensor_scalar with an
    all-ones tile) and one XBAR transpose of [128, 3*SB] per superblock
    yields the partition-broadcast [128, SB] tiles directly.
  - Attention per (batch, head) fully transposed as v3, but the softmax
    rowsum matmul uses an all-ones [128,128] stationary (M=128, not M=1):
    the PSUM result is the rowsum pre-broadcast to all partitions, so the
    normalize is just reciprocal_approx_fast + one tensor_tensor multiply.
    No ScalarE copies and -- critically -- NO GpSimd broadcasts in
    attention, so the AllToAll collectives are not stuck behind attention
    work in the GpSimd queue: A2A(b0) now fires at the start of phase B and
    hides under attention(b1).
  - Output GEMM runs cc-outer over nb-pairs so a stationary cf chunk is
    reused across two 512-wide streams; outG(b0) interleaves into the tail
    of attention(b1) and covers A2A(b1).
"""

import sys

if "/opt/trn_rl_repo" not in sys.path:
    sys.path.insert(0, "/opt/trn_rl_repo")

# --- shim antenv.axon_hooks (missing in this image) so trace=True can NTFF-profile ---
import types, ctypes, contextlib


def _make_ntff_hook(so_path="/opt/axon/libaxon_pjrt.so"):
    try:
        lib = ctypes.CDLL(so_path)
    except OSError:
        return None
    if not hasattr(lib, "axon_start_nrt_profile"):
        return None
    lib.axon_start_nrt_profile.argtypes = [ctypes.POINTER(ctypes.c_int64), ctypes.c_size_t]
    lib.axon_start_nrt_profile.restype = ctypes.c_int64
    lib.axon_stop_nrt_profile.argtypes = [ctypes.c_char_p]
    lib.axon_stop_nrt_profile.restype = ctypes.c_int64

    @contextlib.contextmanager
    def _hook(output_dir, device_ids):
        import jax

        jax.devices()
        if device_ids:
            ids = (ctypes.c_int64 * len(device_ids))(*device_ids)
            rc = lib.axon_start_nrt_profile(ids, len(device_ids))
        else:
            rc = lib.axon_start_nrt_profile(None, 0)
        if rc != 0:
            raise RuntimeError(f"axon_start_nrt_profile rc={rc}")
        try:
            yield
        finally:
            n = lib.axon_stop_nrt_profile(str(output_dir).encode())
            if n < 0:
                raise RuntimeError(f"axon_stop_nrt_profile rc={n}")

    return _hook


if "antenv.axon_hooks" not in sys.modules:
    _m = types.ModuleType("antenv.axon_hooks")
    _m.get_axon_ntff_profile_hook = lambda: _make_ntff_hook()
    sys.modules["antenv.axon_hooks"] = _m
# --- end shim ---

import numpy as np
import ml_dtypes  # noqa: F401  (bf16 numpy dtype registration)

from concourse import bacc, tile, mybir
from concourse.masks import make_upper_triangular

B, S, HID = 2, 2048, 2048
HEADS = 16
HD = 128
T = B * S
N_CORES = 8
HPC = HEADS // N_CORES  # 2 heads per core
EPS = 1e-6
SCALE = 1.0 / float(np.sqrt(HD))

F32 = mybir.dt.float32
BF16 = mybir.dt.bfloat16

SB = 512  # tokens per LN/QKV superblock
N_SB = T // SB  # 8
N_CC = HID // 128  # 16 contraction chunks
TOK_SHARD = S // N_CORES  # 256 tokens per (batch, core) after A2A
QC = 512  # attention q-chunk width
MUL = mybir.AluOpType.mult
SUB = mybir.AluOpType.subtract
ADD = mybir.AluOpType.add


def _build(apply_mask: bool):
    nc = bacc.Bacc("TRN2", target_bir_lowering=False, debug=False, num_devices=N_CORES)

    xt = nc.dram_tensor("xt", [HID, T], BF16, kind="ExternalInput").ap()
    xn = nc.dram_tensor("xn", [T, HID], BF16, kind="ExternalInput").ap()
    wq = nc.dram_tensor("qkvw", [HID, 3 * HPC * HD], BF16, kind="ExternalInput").ap()
    rqk = nc.dram_tensor("rqk", [128, 4], F32, kind="ExternalInput").ap()
    bqd = nc.dram_tensor("bq", [128, HPC], F32, kind="ExternalInput").ap()
    rvd = nc.dram_tensor("rv", [1, HPC * HD], BF16, kind="ExternalInput").ap()
    obd = nc.dram_tensor("obias", [1, HID], BF16, kind="ExternalInput").ap()
    owt = nc.dram_tensor("ow", [HID, HID], BF16, kind="ExternalInput").ap()
    out = nc.dram_tensor("out", [B * TOK_SHARD, HID], F32, kind="ExternalOutput").ap()
    if apply_mask:
        imask = nc.dram_tensor("imask", [128, B * (S // 128)], F32, kind="ExternalInput").ap()

    cc_in = [nc.dram_tensor(f"cc_in{b}", [N_CORES, HPC * HD, TOK_SHARD], BF16).ap() for b in range(B)]
    cc_out = [nc.dram_tensor(f"cc_out{b}", [N_CORES, HPC * HD, TOK_SHARD], BF16).ap() for b in range(B)]

    with tile.TileContext(nc) as tc:
        with tc.tile_pool(name="persist", bufs=1) as pers:
            ones128 = pers.tile([128, 128], BF16)
            nc.gpsimd.memset(ones128[:], 1.0)
            eps_t = pers.tile([128, 1], F32)
            nc.gpsimd.memset(eps_t[:], EPS)
            trif = pers.tile([128, 128], F32)
            make_upper_triangular(nc, trif[:], val=1.0, diag=True)
            tri01 = pers.tile([128, 128], BF16)
            nc.vector.tensor_copy(tri01[:], trif[:])
            rqk_sb = pers.tile([128, 4], F32)
            nc.scalar.dma_start(out=rqk_sb[:], in_=rqk[:])
            bq_sb = pers.tile([128, HPC], F32)
            nc.scalar.dma_start(out=bq_sb[:], in_=bqd[:])
            rv_row = pers.tile([1, HPC * HD], BF16)
            nc.scalar.dma_start(out=rv_row[:], in_=rvd[:])
            rv_bc = pers.tile([128, HPC * HD], BF16)
            nc.gpsimd.partition_broadcast(rv_bc[:], rv_row[:])
            ob_row = pers.tile([1, HID], BF16)
            nc.scalar.dma_start(out=ob_row[:], in_=obd[:])
            ob_bc = pers.tile([128, HID], BF16)
            nc.gpsimd.partition_broadcast(ob_bc[:], ob_row[:])
            if apply_mask:
                msk = pers.tile([128, B * (S // 128)], F32)
                nc.scalar.dma_start(out=msk[:], in_=imask[:])

            qT = pers.tile([128, HPC, T], BF16)  # [d, head, tok]
            kT = pers.tile([128, HPC, T], BF16)
            v_sb = pers.tile([128, T // 128, HPC * HD], BF16)  # [tok128, blk, hcol]
            sistd = pers.tile([128, T // 128], F32)  # SCALE * istd per token-block
            istd_n = pers.tile([128, T // 128], F32)  # 1/sd, natural layout
            c1_n = pers.tile([128, T // 128], F32)  # mu/sd, natural layout

            # ---------- attention emitter (transposed, sw-pipelined) ----------
            def attn_qc(b, qc, ps_mm, ps_ctx, ps_rs, ppT, prs, ctxT):
                nkb = 4 * qc + 4
                ctx_ps = [ps_ctx.tile([128, QC], F32, tag="ctx", name=f"ctx{h}") for h in range(HPC)]
                rs_ps = [ps_rs.tile([128, QC], F32, tag="rs", name=f"rs{h}") for h in range(HPC)]
                kbs = list(range(nkb - 1, -1, -1))
                pend = {}

                def emit_sc(kb):
                    c0 = max(0, (kb - 4 * qc) * 128)
                    w = QC - c0
                    for h in range(HPC):
                        sc = ps_mm.tile([128, QC], F32, tag="mm", name="sc")
                        nc.tensor.matmul(
                            sc[:, :w],
                            kT[:, h, b * S + kb * 128 : b * S + kb * 128 + 128],
                            qT[:, h, b * S + qc * QC + c0 : b * S + qc * QC + c0 + w],
                            start=True,
                            stop=True,
                        )
                        pt = ppT.tile([128, QC], BF16, tag="pt", name="pt")
                        bias = msk[:, b * 16 + kb : b * 16 + kb + 1] if apply_mask else 0.0
                        nc.scalar.activation(
                            pt[:, :w], sc[:, :w], mybir.ActivationFunctionType.Exp,
                            scale=sistd[:, b * 16 + kb : b * 16 + kb + 1], bias=bias,
                        )
                        if kb >= 4 * qc:  # causal diagonal block
                            nc.vector.tensor_mul(pt[:, 0:128], pt[:, 0:128], tri01[:])
                        pend[(h, kb)] = (pt, c0, w)

                def emit_consume(kb):
                    for h in range(HPC):
                        pt, c0, w = pend.pop((h, kb))
                        nc.tensor.matmul(
                            rs_ps[h][:, c0:QC], ones128[:], pt[:, :w],
                            start=(kb == kbs[0]), stop=(kb == 0),
                        )
                        nc.tensor.matmul(
                            ctx_ps[h][:, c0:QC],
                            v_sb[:, b * 16 + kb, h * HD : (h + 1) * HD],
                            pt[:, :w],
                            start=(kb == kbs[0]), stop=(kb == 0),
                        )

                for i, kb in enumerate(kbs):
                    emit_sc(kb)
                    if i > 0:
                        emit_consume(kbs[i - 1])
                emit_consume(kbs[-1])

                for h in range(HPC):
                    rsi = prs.tile([128, QC], F32, tag="rsi", name="rsi")
                    nc.vector.reciprocal_approx_fast(out=rsi[:], in_=rs_ps[h][:])
                    nc.vector.tensor_mul(
                        ctxT[:, h, qc * QC : (qc + 1) * QC], ctx_ps[h][:], rsi[:]
                    )

            def ship_ctx(b, ctxT):
                for j in range(N_CORES):
                    nc.sync.dma_start(
                        out=cc_in[b][j].rearrange("(h d) w -> d h w", d=128),
                        in_=ctxT[:, :, j * TOK_SHARD : (j + 1) * TOK_SHARD],
                    )
                nc.gpsimd.collective_compute(
                    "AllToAll",
                    mybir.AluOpType.bypass,
                    replica_groups=[list(range(N_CORES))],
                    ins=[cc_in[b][:]],
                    outs=[cc_out[b][:]],
                )

            # ---------------- Phase A + attention(b0) interleaved ----------------
            with (
                tc.tile_pool(name="pb_pT", bufs=6) as ppT,
                tc.tile_pool(name="pb_cT", bufs=2) as pcT,
                tc.tile_pool(name="pb_rs_sb", bufs=2) as prs,
                tc.tile_pool(name="ps_mm", bufs=4, space="PSUM") as ps_mm,
                tc.tile_pool(name="ps_ctx", bufs=2, space="PSUM") as ps_ctx,
                tc.tile_pool(name="ps_rs", bufs=2, space="PSUM") as ps_rs,
            ):
                ctxT0 = pcT.tile([128, HPC, S], BF16, tag="ctxT", name="ctxT0")
                with (
                    tc.tile_pool(name="pa_w", bufs=1) as paw,
                    tc.tile_pool(name="pa_xt", bufs=3) as pxt,
                    tc.tile_pool(name="pa_xn", bufs=6) as pxn,
                    tc.tile_pool(name="pa_st", bufs=6) as pst,
                    tc.tile_pool(name="pa_A", bufs=2) as pA,
                    tc.tile_pool(name="pa_bc", bufs=3) as pbc,
                    tc.tile_pool(name="pa_ev", bufs=6) as pe,
                ):
                    w_sb = paw.tile([128, N_CC, 3 * HPC * HD], BF16)
                    for g in range(4):
                        nc.scalar.dma_start(
                            out=w_sb[:, g * 4 : (g + 1) * 4, :],
                            in_=wq[g * 512 : (g + 1) * 512, :].rearrange(
                                "(c p) f -> p c f", p=128
                            ),
                        )

                    def stats_sb(sb):
                        """bn_stats on natural x -> mu/istd/c1.  Each stat is
                        replicated across 128 columns (TS with the all-ones
                        tile), so one XBAR transpose of [128, 3*SB] yields the
                        partition-broadcast [128, SB] tiles directly:
                        bcast[:, s*4+tb, :] = stat_s(block tb) in every row."""
                        srep = pA.tile([128, 3 * SB], BF16, tag="A", name="srep")
                        for tb in range(4):
                            r0 = sb * SB + tb * 128
                            x_t = pxn.tile([128, HID], BF16, tag="xn", name="x_t")
                            nc.sync.dma_start(out=x_t[:], in_=xn[r0 : r0 + 128, :])
                            bn = pst.tile([128, 4, 6], F32, tag="bn", name="bn")
                            for c4 in range(4):
                                nc.vector.bn_stats(bn[:, c4, :], x_t[:, c4 * 512 : (c4 + 1) * 512])
                            mv = pst.tile([128, 2], F32, tag="mv", name="mv")
                            nc.vector.bn_aggr(mv[:], bn[:])
                            sd = pst.tile([128, 1], F32, tag="sd", name="sd")
                            nc.scalar.activation(
                                sd[:], mv[:, 1:2], mybir.ActivationFunctionType.Sqrt, bias=eps_t[:]
                            )
                            istd = pst.tile([128, 1], F32, tag="istd", name="istd")
                            nc.vector.reciprocal_approx_fast(out=istd[:], in_=sd[:])
                            blk = sb * 4 + tb
                            nc.vector.tensor_scalar(
                                out=sistd[:, blk : blk + 1], in0=istd[:],
                                scalar1=SCALE, scalar2=None, op0=MUL,
                            )
                            nc.vector.tensor_copy(istd_n[:, blk : blk + 1], istd[:])
                            nc.vector.tensor_scalar(
                                out=c1_n[:, blk : blk + 1], in0=mv[:, 0:1],
                                scalar1=istd[:], scalar2=None, op0=MUL,
                            )
                            cs = slice(0 * SB + tb * 128, 0 * SB + (tb + 1) * 128)
                            nc.vector.tensor_scalar(
                                out=srep[:, cs], in0=ones128[:],
                                scalar1=mv[:, 0:1], scalar2=None, op0=MUL,
                            )
                            cs = slice(1 * SB + tb * 128, 1 * SB + (tb + 1) * 128)
                            nc.vector.tensor_scalar(
                                out=srep[:, cs], in0=ones128[:],
                                scalar1=istd[:], scalar2=None, op0=MUL,
                            )
                            cs = slice(2 * SB + tb * 128, 2 * SB + (tb + 1) * 128)
                            nc.vector.tensor_scalar(
                                out=srep[:, cs], in0=ones128[:],
                                scalar1=c1_n[:, blk : blk + 1], scalar2=None, op0=MUL,
                            )
                        bc = pbc.tile([128, 12, 128], BF16, tag="bc", name="bc")
                        nc.scalar.dma_start_transpose(out=bc[:], in_=srep[:])
                        flat = bc[:].rearrange("p a b -> p (a b)")
                        mu_bc = flat[:, 0 * SB : 1 * SB]
                        istd_bc = flat[:, 1 * SB : 2 * SB]
                        c1_bc = flat[:, 2 * SB : 3 * SB]
                        return mu_bc, istd_bc, c1_bc

                    def load_xt(sb):
                        """Split into 4-cc groups so the first chain matmuls can
                        start as soon as the first 0.5MB lands."""
                        col0 = sb * SB
                        xt_t = pxt.tile([128, N_CC, SB], BF16, tag="xt", name="xt_t")
                        for g in range(4):
                            nc.sync.dma_start(
                                out=xt_t[:, g * 4 : (g + 1) * 4, :],
                                in_=xt[g * 512 : (g + 1) * 512, col0 : col0 + SB].rearrange(
                                    "(c p) t -> p c t", p=128
                                ),
                            )
                        return xt_t

                    def qkv_sb(sb, bcs, xt_t):
                        mu_bc, istd_bc, c1_bc = bcs
                        col0 = sb * SB
                        for h in range(HPC):
                            psq = ps_mm.tile([128, SB], F32, tag="mm", name="psq")
                            for cc in range(N_CC):
                                nc.tensor.matmul(
                                    psq[:],
                                    w_sb[:, cc, h * HD : (h + 1) * HD],
                                    xt_t[:, cc, :],
                                    start=(cc == 0),
                                    stop=(cc == N_CC - 1),
                                )
                            t1 = pe.tile([128, SB], BF16, tag="ev", name="t1")
                            nc.vector.tensor_mul(t1[:], psq[:], istd_bc)
                            # t2 = c1*rq - bq  (bq negated on host)
                            t2 = pe.tile([128, SB], BF16, tag="ev", name="t2")
                            nc.vector.tensor_scalar(
                                out=t2[:], in0=c1_bc,
                                scalar1=rqk_sb[:, h : h + 1], scalar2=bq_sb[:, h : h + 1],
                                op0=MUL, op1=ADD,
                            )
                            nc.vector.tensor_sub(qT[:, h, col0 : col0 + SB], t1[:], t2[:])

                            psk = ps_mm.tile([128, SB], F32, tag="mm", name="psk")
                            for cc in range(N_CC):
                                nc.tensor.matmul(
                                    psk[:],
                                    w_sb[:, cc, HPC * HD + h * HD : HPC * HD + (h + 1) * HD],
                                    xt_t[:, cc, :],
                                    start=(cc == 0),
                                    stop=(cc == N_CC - 1),
                                )
                            # k = psk - mu*rk  (rk negated on host; istd_k folded
                            # into the exp scale)
                            nc.vector.scalar_tensor_tensor(
                                out=kT[:, h, col0 : col0 + SB],
                                in0=mu_bc, scalar=rqk_sb[:, 2 + h : 3 + h], in1=psk[:],
                                op0=MUL, op1=ADD,
                            )
                        for tb2 in range(4):
                            psv = ps_mm.tile([128, HPC * HD], F32, tag="mm", name="psv")
                            for cc in range(N_CC):
                                nc.tensor.matmul(
                                    psv[:],
                                    xt_t[:, cc, tb2 * 128 : (tb2 + 1) * 128],
                                    w_sb[:, cc, 2 * HPC * HD :],
                                    start=(cc == 0),
                                    stop=(cc == N_CC - 1),
                                )
                            blk = sb * 4 + tb2
                            va = pe.tile([128, HPC * HD], BF16, tag="ev", name="va")
                            nc.vector.tensor_scalar(
                                out=va[:], in0=psv[:],
                                scalar1=istd_n[:, blk : blk + 1], scalar2=None, op0=MUL,
                            )
                            # v = va - c1*rv  (rv negated on host)
                            nc.vector.scalar_tensor_tensor(
                                out=v_sb[:, blk, :],
                                in0=rv_bc[:], scalar=c1_n[:, blk : blk + 1], in1=va[:],
                                op0=MUL, op1=ADD,
                            )

                    bcs = [None] * N_SB
                    xts = [None] * N_SB
                    bcs[0] = stats_sb(0)
                    xts[0] = load_xt(0)
                    bcs[1] = stats_sb(1)
                    xts[1] = load_xt(1)
                    qkv_sb(0, bcs[0], xts[0])
                    bcs[2] = stats_sb(2)
                    xts[2] = load_xt(2)
                    qkv_sb(1, bcs[1], xts[1])
                    bcs[3] = stats_sb(3)
                    xts[3] = load_xt(3)
                    qkv_sb(2, bcs[2], xts[2])
                    bcs[4] = stats_sb(4)
                    xts[4] = load_xt(4)
                    qkv_sb(3, bcs[3], xts[3])
                    bcs[5] = stats_sb(5)
                    xts[5] = load_xt(5)
                    qkv_sb(4, bcs[4], xts[4])
                    attn_qc(0, 0, ps_mm, ps_ctx, ps_rs, ppT, prs, ctxT0)
                    bcs[6] = stats_sb(6)
                    xts[6] = load_xt(6)
                    qkv_sb(5, bcs[5], xts[5])
                    attn_qc(0, 1, ps_mm, ps_ctx, ps_rs, ppT, prs, ctxT0)
                    bcs[7] = stats_sb(7)
                    xts[7] = load_xt(7)
                    qkv_sb(6, bcs[6], xts[6])
                    attn_qc(0, 2, ps_mm, ps_ctx, ps_rs, ppT, prs, ctxT0)
                    qkv_sb(7, bcs[7], xts[7])
                    attn_qc(0, 3, ps_mm, ps_ctx, ps_rs, ppT, prs, ctxT0)
                    ship_ctx(0, ctxT0)

                # ---------- Phase B: attention(b1) + output GEMMs ----------
                with (
                    tc.tile_pool(name="pb_ow", bufs=1) as pow_,
                    tc.tile_pool(name="pb_cf", bufs=2) as pcf,
                    tc.tile_pool(name="pb_o", bufs=2) as po,
                ):
                    ow_sb = pow_.tile([128, N_CC, HID], BF16)
                    for g in range(4):
                        nc.scalar.dma_start(
                            out=ow_sb[:, g * 4 : (g + 1) * 4, :],
                            in_=owt[g * 512 : (g + 1) * 512, :].rearrange(
                                "(c p) f -> p c f", p=128
                            ),
                        )
                    def load_cf(b):
                        cf = pcf.tile([128, N_CC, TOK_SHARD], BF16, tag="cf", name=f"cf{b}")
                        src = cc_out[b].rearrange("j (h d) w -> d (j h) w", d=128)
                        nc.sync.dma_start(out=cf[:, 0:8, :], in_=src[:, 0:8, :])
                        nc.sync.dma_start(out=cf[:, 8:16, :], in_=src[:, 8:16, :])
                        return cf

                    cf0 = load_cf(0)

                    def outg_tb(b, cf, tb):
                        o_t = po.tile([128, HID], F32, tag="o", name="o_t")
                        for half in range(2):
                            pso = [
                                ps_mm.tile([128, 512], F32, tag="mm", name=f"pso{nb}")
                                for nb in (2 * half, 2 * half + 1)
                            ]
                            for cc in range(N_CC):
                                for i, nb in enumerate((2 * half, 2 * half + 1)):
                                    nc.tensor.matmul(
                                        pso[i][:],
                                        cf[:, cc, tb * 128 : (tb + 1) * 128],
                                        ow_sb[:, cc, nb * 512 : (nb + 1) * 512],
                                        start=(cc == 0),
                                        stop=(cc == N_CC - 1),
                                    )
                            for i, nb in enumerate((2 * half, 2 * half + 1)):
                                nc.vector.tensor_add(
                                    o_t[:, nb * 512 : (nb + 1) * 512], pso[i][:],
                                    ob_bc[:, nb * 512 : (nb + 1) * 512],
                                )
                            nc.sync.dma_start(
                                out=out[
                                    b * TOK_SHARD + tb * 128 : b * TOK_SHARD + (tb + 1) * 128,
                                    half * 1024 : (half + 1) * 1024,
                                ],
                                in_=o_t[:, half * 1024 : (half + 1) * 1024],
                            )

                    ctxT1 = pcT.tile([128, HPC, S], BF16, tag="ctxT", name="ctxT1")
                    attn_qc(1, 0, ps_mm, ps_ctx, ps_rs, ppT, prs, ctxT1)
                    attn_qc(1, 1, ps_mm, ps_ctx, ps_rs, ppT, prs, ctxT1)
                    attn_qc(1, 2, ps_mm, ps_ctx, ps_rs, ppT, prs, ctxT1)
                    attn_qc(1, 3, ps_mm, ps_ctx, ps_rs, ppT, prs, ctxT1)
                    ship_ctx(1, ctxT1)
                    # outG(b0) fills the PE while A2A(b1) is in flight
                    outg_tb(0, cf0, 0)
                    cf1 = load_cf(1)
                    outg_tb(0, cf0, 1)
                    outg_tb(1, cf1, 0)
                    outg_tb(1, cf1, 1)

    nc.compile()
    return nc


_CACHE = {}


def _get_nc(apply_mask: bool):
    if apply_mask not in _CACHE:
        _CACHE[apply_mask] = _build(apply_mask)
    return _CACHE[apply_mask]


def _prep_in_maps(input, input_mask, norm_w, norm_b, attn_qkvw, attn_qkvb, attn_ow):
    bf16 = ml_dtypes.bfloat16
    x = np.asarray(input, dtype=np.float32).reshape(T, HID)
    w = np.asarray(attn_qkvw, dtype=np.float32)
    nw = np.asarray(norm_w, dtype=np.float32)
    nb = np.asarray(norm_b, dtype=np.float32)
    qb_ = np.asarray(attn_qkvb, dtype=np.float32)
    ow_f = np.asarray(attn_ow, dtype=np.float32)
    ow = np.ascontiguousarray(ow_f.astype(bf16))
    mask = np.asarray(input_mask, dtype=np.float32).reshape(B, S)

    xt = np.ascontiguousarray(x.T.astype(bf16))  # [HID, T]
    xn = np.ascontiguousarray(x.astype(bf16))  # [T, HID]

    w_eff = nw[:, None] * w  # fold LN gamma into QKV weight
    b_eff = nb @ w + qb_  # fold LN beta into QKV bias
    colsum = w_eff.sum(axis=0)  # [3*HID]
    obias_full = np.ascontiguousarray(
        (b_eff[2 * HID :] @ ow_f).reshape(1, HID).astype(bf16)
    )

    apply_mask = bool(np.any(mask != 0.0))
    if apply_mask:
        # per-key layout: [128 partitions (k within block), B * 16 key-blocks]
        mprep = np.ascontiguousarray(
            mask.reshape(B, S // 128, 128).transpose(2, 0, 1).reshape(128, B * (S // 128))
        )
    in_maps = []
    for i in range(N_CORES):
        cols = []
        for part in range(3):  # q, k, v column shards for this core's heads
            c0 = part * HID + i * HPC * HD
            cols.append(w_eff[:, c0 : c0 + HPC * HD])
        wqkv_i = np.ascontiguousarray(np.concatenate(cols, axis=1).astype(bf16))

        q0 = i * HPC * HD
        k0 = HID + i * HPC * HD
        v0 = 2 * HID + i * HPC * HD
        rqk_i = np.ascontiguousarray(
            np.stack(
                [
                    colsum[q0 : q0 + HD],
                    colsum[q0 + HD : q0 + 2 * HD],
                    -colsum[k0 : k0 + HD],
                    -colsum[k0 + HD : k0 + 2 * HD],
                ],
                axis=1,
            ).astype(np.float32)
        )
        # negated: the ScalarE Identity epilogue computes c1*rq + (-bq)
        bq_i = np.ascontiguousarray(
            np.stack(
                [-b_eff[q0 : q0 + HD], -b_eff[q0 + HD : q0 + 2 * HD]], axis=1
            ).astype(np.float32)
        )
        rv_i = np.ascontiguousarray(
            (-colsum[v0 : v0 + HPC * HD]).reshape(1, HPC * HD).astype(bf16)
        )
        m = {
            "xt": xt,
            "xn": xn,
            "qkvw": wqkv_i,
            "rqk": rqk_i,
            "bq": bq_i,
            "rv": rv_i,
            "obias": obias_full,
            "ow": ow,
        }
        if apply_mask:
            m["imask"] = mprep
        in_maps.append(m)
    return in_maps, apply_mask


def _run(inputs: dict, trace: bool = False):
    from concourse.bass_utils import run_bass_kernel_spmd

    in_maps, apply_mask = _prep_in_maps(**inputs)
    nc = _get_nc(apply_mask)
    res = run_bass_kernel_spmd(nc, in_maps, list(range(N_CORES)), trace=trace)
    out = np.empty((B, S, HID), dtype=np.float32)
    for j in range(N_CORES):
        o = res.results[j]["out"]
        for b in range(B):
            out[b, j * TOK_SHARD : (j + 1) * TOK_SHARD] = o[b * TOK_SHARD : (b + 1) * TOK_SHARD]
    return out, res


def kernel(**inputs) -> np.ndarray:
    out, _ = _run(inputs, trace=False)
    return out


# revision 33
# speedup vs baseline: 1.2544x; 1.0095x over previous
"""Tensor-parallel DeepSpeed-style self-attention block on 8 TRN2 NeuronCores.

v4 strategy (fold LN into GEMM epilogues; kill startup + queue serialization):
  - Host pre-transposes the input to xT (bf16) and also passes a natural
    bf16 copy for LayerNorm statistics.  The QKV GEMM runs directly on the
    RAW xT -- no z pass, no on-device zT transposes -- so the first matmul
    fires as soon as the first weight/xT chunks land (~10us vs ~67us).
  - LN is applied algebraically in the PSUM evacuation:
        q = istd .* psq - (mu*istd) .* rq + bq         (DVE, 3 ops)
        k = psk - mu .* rk                             (DVE STT, 1 op;
          the istd_k factor rides for free in the exp's per-partition scale)
        v = istd .* (psv - mu .* rv)                   (DVE, 2 ops;
          the V bias is exact via softmax-rows-sum-to-1 and is folded into a
          host-precomputed output bias added in the output-GEMM epilogue)
    where rq/rk/rv are host-precomputed column sums of the LN-folded QKV
    weight and mu/istd come from on-device bn_stats over the natural copy.
    Each stat is replicated across 128 columns (tensor_scalar with an
    all-ones tile) and one XBAR transpose of [128, 3*SB] per superblock
    yields the partition-broadcast [128, SB] tiles directly.
  - Attention per (batch, head) fully transposed as v3, but the softmax
    rowsum matmul uses an all-ones [128,128] stationary (M=128, not M=1):
    the PSUM result is the rowsum pre-broadcast to all partitions, so the
    normalize is just reciprocal_approx_fast + one tensor_tensor multiply.
    No ScalarE copies and -- critically -- NO GpSimd broadcasts in
    attention, so the AllToAll collectives are not stuck behind attention
    work in the GpSimd queue: A2A(b0) now fires at the start of phase B and
    hides under attention(b1).
  - Output GEMM runs cc-outer over nb-pairs so a stationary cf chunk is
    reused across two 512-wide streams; outG(b0) interleaves into the tail
    of attention(b1) and covers A2A(b1).
"""

import sys

if "/opt/trn_rl_repo" not in sys.path:
    sys.path.insert(0, "/opt/trn_rl_repo")

# --- shim antenv.axon_hooks (missing in this image) so trace=True can NTFF-profile ---
import types, ctypes, contextlib


def _make_ntff_hook(so_path="/opt/axon/libaxon_pjrt.so"):
    try:
        lib = ctypes.CDLL(so_path)
    except OSError:
        return None
    if not hasattr(lib, "axon_start_nrt_profile"):
        return None
    lib.axon_start_nrt_profile.argtypes = [ctypes.POINTER(ctypes.c_int64), ctypes.c_size_t]
    lib.axon_start_nrt_profile.restype = ctypes.c_int64
    lib.axon_stop_nrt_profile.argtypes = [ctypes.c_char_p]
    lib.axon_stop_nrt_profile.restype = ctypes.c_int64

    @contextlib.contextmanager
    def _hook(output_dir, device_ids):
        import jax

        jax.devices()
        if device_ids:
            ids = (ctypes.c_int64 * len(device_ids))(*device_ids)
            rc = lib.axon_start_nrt_profile(ids, len(device_ids))
        else:
            rc = lib.axon_start_nrt_profile(None, 0)
        if rc != 0:
            raise RuntimeError(f"axon_start_nrt_profile rc={rc}")
        try:
            yield
        finally:
            n = lib.axon_stop_nrt_profile(str(output_dir).encode())
            if n < 0:
                raise RuntimeError(f"axon_stop_nrt_profile rc={n}")

    return _hook


if "antenv.axon_hooks" not in sys.modules:
    _m = types.ModuleType("antenv.axon_hooks")
    _m.get_axon_ntff_profile_hook = lambda: _make_ntff_hook()
    sys.modules["antenv.axon_hooks"] = _m
# --- end shim ---

import numpy as np
import ml_dtypes  # noqa: F401  (bf16 numpy dtype registration)

from concourse import bacc, tile, mybir
from concourse.masks import make_upper_triangular

B, S, HID = 2, 2048, 2048
HEADS = 16
HD = 128
T = B * S
N_CORES = 8
HPC = HEADS // N_CORES  # 2 heads per core
EPS = 1e-6
SCALE = 1.0 / float(np.sqrt(HD))

F32 = mybir.dt.float32
BF16 = mybir.dt.bfloat16

SB = 512  # tokens per LN/QKV superblock
N_SB = T // SB  # 8
N_CC = HID // 128  # 16 contraction chunks
TOK_SHARD = S // N_CORES  # 256 tokens per (batch, core) after A2A
QC = 512  # attention q-chunk width
MUL = mybir.AluOpType.mult
SUB = mybir.AluOpType.subtract
ADD = mybir.AluOpType.add


def _build(apply_mask: bool):
    nc = bacc.Bacc("TRN2", target_bir_lowering=False, debug=False, num_devices=N_CORES)

    xt = nc.dram_tensor("xt", [N_SB * 128, N_CC * SB], BF16, kind="ExternalInput").ap()
    xn = nc.dram_tensor("xn", [T, HID], BF16, kind="ExternalInput").ap()
    wq = nc.dram_tensor("qkvw", [128, N_CC * 3 * HPC * HD], BF16, kind="ExternalInput").ap()
    rqk = nc.dram_tensor("rqk", [128, 4], F32, kind="ExternalInput").ap()
    bqd = nc.dram_tensor("bq", [128, HPC], F32, kind="ExternalInput").ap()
    rvd = nc.dram_tensor("rv", [1, HPC * HD], BF16, kind="ExternalInput").ap()
    obd = nc.dram_tensor("obias", [1, HID], BF16, kind="ExternalInput").ap()
    owt = nc.dram_tensor("ow", [128, N_CC * HID], BF16, kind="ExternalInput").ap()
    out = nc.dram_tensor("out", [B * TOK_SHARD, HID], F32, kind="ExternalOutput").ap()
    if apply_mask:
        imask = nc.dram_tensor("imask", [128, B * (S // 128)], F32, kind="ExternalInput").ap()

    cc_in = [nc.dram_tensor(f"cc_in{b}", [N_CORES, 128, HPC * TOK_SHARD], BF16).ap() for b in range(B)]
    cc_out = [nc.dram_tensor(f"cc_out{b}", [N_CORES, 128, HPC * TOK_SHARD], BF16).ap() for b in range(B)]

    with tile.TileContext(nc) as tc:
        with tc.tile_pool(name="persist", bufs=1) as pers:
            ones128 = pers.tile([128, 128], BF16)
            nc.gpsimd.memset(ones128[:], 1.0)
            eps_t = pers.tile([128, 1], F32)
            nc.gpsimd.memset(eps_t[:], EPS)
            trif = pers.tile([128, 128], F32)
            make_upper_triangular(nc, trif[:], val=1.0, diag=True)
            tri01 = pers.tile([128, 128], BF16)
            nc.vector.tensor_copy(tri01[:], trif[:])
            rqk_sb = pers.tile([128, 4], F32)
            nc.scalar.dma_start(out=rqk_sb[:], in_=rqk[:])
            bq_sb = pers.tile([128, HPC], F32)
            nc.scalar.dma_start(out=bq_sb[:], in_=bqd[:])
            rv_row = pers.tile([1, HPC * HD], BF16)
            nc.scalar.dma_start(out=rv_row[:], in_=rvd[:])
            rv_bc = pers.tile([128, HPC * HD], BF16)
            nc.gpsimd.partition_broadcast(rv_bc[:], rv_row[:])
            ob_row = pers.tile([1, HID], BF16)
            nc.scalar.dma_start(out=ob_row[:], in_=obd[:])
            ob_bc = pers.tile([128, HID], BF16)
            nc.gpsimd.partition_broadcast(ob_bc[:], ob_row[:])
            if apply_mask:
                msk = pers.tile([128, B * (S // 128)], F32)
                nc.scalar.dma_start(out=msk[:], in_=imask[:])

            qT = pers.tile([128, HPC, T], BF16)  # [d, head, tok]
            kT = pers.tile([128, HPC, T], BF16)
            v_sb = pers.tile([128, T // 128, HPC * HD], BF16)  # [tok128, blk, hcol]
            sistd = pers.tile([128, T // 128], F32)  # SCALE * istd per token-block
            istd_n = pers.tile([128, T // 128], F32)  # 1/sd, natural layout
            c1_n = pers.tile([128, T // 128], F32)  # mu/sd, natural layout

            # ---------- attention emitter (transposed, sw-pipelined) ----------
            def attn_qc(b, qc, ps_mm, ps_ctx, ps_rs, ppT, prs, ctxT):
                nkb = 4 * qc + 4
                ctx_ps = [ps_ctx.tile([128, QC], F32, tag="ctx", name=f"ctx{h}") for h in range(HPC)]
                rs_ps = [ps_rs.tile([128, QC], F32, tag="rs", name=f"rs{h}") for h in range(HPC)]
                kbs = list(range(nkb - 1, -1, -1))
                pend = {}

                def emit_sc(kb):
                    c0 = max(0, (kb - 4 * qc) * 128)
                    w = QC - c0
                    for h in range(HPC):
                        sc = ps_mm.tile([128, QC], F32, tag="mm", name="sc")
                        nc.tensor.matmul(
                            sc[:, :w],
                            kT[:, h, b * S + kb * 128 : b * S + kb * 128 + 128],
                            qT[:, h, b * S + qc * QC + c0 : b * S + qc * QC + c0 + w],
                            start=True,
                            stop=True,
                        )
                        pt = ppT.tile([128, QC], BF16, tag="pt", name="pt")
                        bias = msk[:, b * 16 + kb : b * 16 + kb + 1] if apply_mask else 0.0
                        nc.scalar.activation(
                            pt[:, :w], sc[:, :w], mybir.ActivationFunctionType.Exp,
                            scale=sistd[:, b * 16 + kb : b * 16 + kb + 1], bias=bias,
                        )
                        if kb >= 4 * qc:  # causal diagonal block
                            nc.vector.tensor_mul(pt[:, 0:128], pt[:, 0:128], tri01[:])
                        pend[(h, kb)] = (pt, c0, w)

                def emit_consume(kb):
                    for h in range(HPC):
                        pt, c0, w = pend.pop((h, kb))
                        nc.tensor.matmul(
                            rs_ps[h][:, c0:QC], ones128[:], pt[:, :w],
                            start=(kb == kbs[0]), stop=(kb == 0),
                        )
                        nc.tensor.matmul(
                            ctx_ps[h][:, c0:QC],
                            v_sb[:, b * 16 + kb, h * HD : (h + 1) * HD],
                            pt[:, :w],
                            start=(kb == kbs[0]), stop=(kb == 0),
                        )

                for i, kb in enumerate(kbs):
                    emit_sc(kb)
                    if i > 1:
                        emit_consume(kbs[i - 2])
                emit_consume(kbs[-2])
                emit_consume(kbs[-1])

                for h in range(HPC):
                    rsi = prs.tile([128, QC], F32, tag="rsi", name="rsi")
                    nc.vector.reciprocal_approx_fast(out=rsi[:], in_=rs_ps[h][:])
                    nc.vector.tensor_mul(
                        ctxT[:, h, qc * QC : (qc + 1) * QC], ctx_ps[h][:], rsi[:]
                    )

            def ship_qc(b, qc, ctxT):
                # dest shards 2*qc, 2*qc+1 are complete once q-chunk qc is done
                for j in (2 * qc, 2 * qc + 1):
                    nc.sync.dma_start(
                        out=cc_in[b][j],
                        in_=ctxT[:, :, j * TOK_SHARD : (j + 1) * TOK_SHARD],
                    )

            def ship_ctx(b, ctxT):
                nc.gpsimd.collective_compute(
                    "AllToAll",
                    mybir.AluOpType.bypass,
                    replica_groups=[list(range(N_CORES))],
                    ins=[cc_in[b][:]],
                    outs=[cc_out[b][:]],
                )

            # ---------------- Phase A + attention(b0) interleaved ----------------
            with (
                tc.tile_pool(name="pb_pT", bufs=8) as ppT,
                tc.tile_pool(name="pb_cT", bufs=2) as pcT,
                tc.tile_pool(name="pb_rs_sb", bufs=2) as prs,
                tc.tile_pool(name="ps_mm", bufs=4, space="PSUM") as ps_mm,
                tc.tile_pool(name="ps_ctx", bufs=2, space="PSUM") as ps_ctx,
                tc.tile_pool(name="ps_rs", bufs=2, space="PSUM") as ps_rs,
            ):
                ctxT0 = pcT.tile([128, HPC, S], BF16, tag="ctxT", name="ctxT0")
                with (
                    tc.tile_pool(name="pa_w", bufs=1) as paw,
                    tc.tile_pool(name="pa_xt", bufs=2) as pxt,
                    tc.tile_pool(name="pa_xn", bufs=8) as pxn,
                    tc.tile_pool(name="pa_st", bufs=6) as pst,
                    tc.tile_pool(name="pa_A", bufs=2) as pA,
                    tc.tile_pool(name="pa_bc", bufs=3) as pbc,
                    tc.tile_pool(name="pa_ev", bufs=6) as pe,
                ):
                    w_sb = paw.tile([128, N_CC, 3 * HPC * HD], BF16)
                    WG = 4 * 3 * HPC * HD
                    for g in range(4):
                        nc.scalar.dma_start(
                            out=w_sb[:, g * 4 : (g + 1) * 4, :],
                            in_=wq[:, g * WG : (g + 1) * WG],
                        )

                    def stats_sb(sb):
                        """bn_stats on natural x -> mu/istd/c1.  Each stat is
                        replicated across 128 columns (TS with the all-ones
                        tile), so one XBAR transpose of [128, 3*SB] yields the
                        partition-broadcast [128, SB] tiles directly:
                        bcast[:, s*4+tb, :] = stat_s(block tb) in every row."""
                        srep = pA.tile([128, 3 * SB], BF16, tag="A", name="srep")
                        for tb in range(4):
                            r0 = sb * SB + tb * 128
                            x_t = pxn.tile([128, HID], BF16, tag="xn", name="x_t")
                            nc.sync.dma_start(out=x_t[:], in_=xn[r0 : r0 + 128, :])
                            bn = pst.tile([128, 4, 6], F32, tag="bn", name="bn")
                            for c4 in range(4):
                                nc.vector.bn_stats(bn[:, c4, :], x_t[:, c4 * 512 : (c4 + 1) * 512])
                            mv = pst.tile([128, 2], F32, tag="mv", name="mv")
                            nc.vector.bn_aggr(mv[:], bn[:])
                            sd = pst.tile([128, 1], F32, tag="sd", name="sd")
                            nc.scalar.activation(
                                sd[:], mv[:, 1:2], mybir.ActivationFunctionType.Sqrt, bias=eps_t[:]
                            )
                            istd = pst.tile([128, 1], F32, tag="istd", name="istd")
                            nc.vector.reciprocal_approx_fast(out=istd[:], in_=sd[:])
                            blk = sb * 4 + tb
                            nc.vector.tensor_scalar(
                                out=sistd[:, blk : blk + 1], in0=istd[:],
                                scalar1=SCALE, scalar2=None, op0=MUL,
                            )
                            nc.vector.tensor_copy(istd_n[:, blk : blk + 1], istd[:])
                            nc.vector.tensor_scalar(
                                out=c1_n[:, blk : blk + 1], in0=mv[:, 0:1],
                                scalar1=istd[:], scalar2=None, op0=MUL,
                            )
                            cs = slice(0 * SB + tb * 128, 0 * SB + (tb + 1) * 128)
                            nc.vector.tensor_scalar(
                                out=srep[:, cs], in0=ones128[:],
                                scalar1=mv[:, 0:1], scalar2=None, op0=MUL,
                            )
                            cs = slice(1 * SB + tb * 128, 1 * SB + (tb + 1) * 128)
                            nc.vector.tensor_scalar(
                                out=srep[:, cs], in0=ones128[:],
                                scalar1=istd[:], scalar2=None, op0=MUL,
                            )
                            cs = slice(2 * SB + tb * 128, 2 * SB + (tb + 1) * 128)
                            nc.vector.tensor_scalar(
                                out=srep[:, cs], in0=ones128[:],
                                scalar1=c1_n[:, blk : blk + 1], scalar2=None, op0=MUL,
                            )
                        bc = pbc.tile([128, 12, 128], BF16, tag="bc", name="bc")
                        nc.scalar.dma_start_transpose(out=bc[:], in_=srep[:])
                        flat = bc[:].rearrange("p a b -> p (a b)")
                        mu_bc = flat[:, 0 * SB : 1 * SB]
                        istd_bc = flat[:, 1 * SB : 2 * SB]
                        c1_bc = flat[:, 2 * SB : 3 * SB]
                        return mu_bc, istd_bc, c1_bc

                    def load_xt(sb):
                        """Split into 4-cc groups so the first chain matmuls can
                        start as soon as the first 0.5MB lands."""
                        xt_t = pxt.tile([128, N_CC, SB], BF16, tag="xt", name="xt_t")
                        rows = slice(sb * 128, (sb + 1) * 128)
                        for g in range(4):
                            nc.sync.dma_start(
                                out=xt_t[:, g * 4 : (g + 1) * 4, :],
                                in_=xt[rows, g * 4 * SB : (g + 1) * 4 * SB],
                            )
                        return xt_t

                    def qkv_sb(sb, bcs, xt_t):
                        mu_bc, istd_bc, c1_bc = bcs
                        col0 = sb * SB
                        for h in range(HPC):
                            psq = ps_mm.tile([128, SB], F32, tag="mm", name="psq")
                            for cc in range(N_CC):
                                nc.tensor.matmul(
                                    psq[:],
                                    w_sb[:, cc, h * HD : (h + 1) * HD],
                                    xt_t[:, cc, :],
                                    start=(cc == 0),
                                    stop=(cc == N_CC - 1),
                                )
                            t1 = pe.tile([128, SB], BF16, tag="ev", name="t1")
                            nc.vector.tensor_mul(t1[:], psq[:], istd_bc)
                            # t2 = c1*rq - bq  (bq negated on host)
                            t2 = pe.tile([128, SB], BF16, tag="ev", name="t2")
                            nc.vector.tensor_scalar(
                                out=t2[:], in0=c1_bc,
                                scalar1=rqk_sb[:, h : h + 1], scalar2=bq_sb[:, h : h + 1],
                                op0=MUL, op1=ADD,
                            )
                            nc.vector.tensor_sub(qT[:, h, col0 : col0 + SB], t1[:], t2[:])

                            psk = ps_mm.tile([128, SB], F32, tag="mm", name="psk")
                            for cc in range(N_CC):
                                nc.tensor.matmul(
                                    psk[:],
                                    w_sb[:, cc, HPC * HD + h * HD : HPC * HD + (h + 1) * HD],
                                    xt_t[:, cc, :],
                                    start=(cc == 0),
                                    stop=(cc == N_CC - 1),
                                )
                            # k = psk - mu*rk  (rk negated on host; istd_k folded
                            # into the exp scale)
                            nc.vector.scalar_tensor_tensor(
                                out=kT[:, h, col0 : col0 + SB],
                                in0=mu_bc, scalar=rqk_sb[:, 2 + h : 3 + h], in1=psk[:],
                                op0=MUL, op1=ADD,
                            )
                        for tb2 in range(4):
                            psv = ps_mm.tile([128, HPC * HD], F32, tag="mm", name="psv")
                            for cc in range(N_CC):
                                nc.tensor.matmul(
                                    psv[:],
                                    xt_t[:, cc, tb2 * 128 : (tb2 + 1) * 128],
                                    w_sb[:, cc, 2 * HPC * HD :],
                                    start=(cc == 0),
                                    stop=(cc == N_CC - 1),
                                )
                            blk = sb * 4 + tb2
                            va = pe.tile([128, HPC * HD], BF16, tag="ev", name="va")
                            nc.vector.tensor_scalar(
                                out=va[:], in0=psv[:],
                                scalar1=istd_n[:, blk : blk + 1], scalar2=None, op0=MUL,
                            )
                            # v = va - c1*rv  (rv negated on host)
                            nc.vector.scalar_tensor_tensor(
                                out=v_sb[:, blk, :],
                                in0=rv_bc[:], scalar=c1_n[:, blk : blk + 1], in1=va[:],
                                op0=MUL, op1=ADD,
                            )

                    bcs = [None] * N_SB
                    xts = [None] * N_SB
                    xts[0] = load_xt(0)
                    bcs[0] = stats_sb(0)
                    bcs[1] = stats_sb(1)
                    xts[1] = load_xt(1)
                    qkv_sb(0, bcs[0], xts[0])
                    bcs[2] = stats_sb(2)
                    xts[2] = load_xt(2)
                    qkv_sb(1, bcs[1], xts[1])
                    bcs[3] = stats_sb(3)
                    xts[3] = load_xt(3)
                    qkv_sb(2, bcs[2], xts[2])
                    bcs[4] = stats_sb(4)
                    xts[4] = load_xt(4)
                    qkv_sb(3, bcs[3], xts[3])
                    bcs[5] = stats_sb(5)
                    xts[5] = load_xt(5)
                    qkv_sb(4, bcs[4], xts[4])
                    attn_qc(0, 0, ps_mm, ps_ctx, ps_rs, ppT, prs, ctxT0)
                    bcs[6] = stats_sb(6)
                    xts[6] = load_xt(6)
                    qkv_sb(5, bcs[5], xts[5])
                    attn_qc(0, 1, ps_mm, ps_ctx, ps_rs, ppT, prs, ctxT0)
                    bcs[7] = stats_sb(7)
                    xts[7] = load_xt(7)
                    qkv_sb(6, bcs[6], xts[6])
                    attn_qc(0, 2, ps_mm, ps_ctx, ps_rs, ppT, prs, ctxT0)
                    qkv_sb(7, bcs[7], xts[7])
                    attn_qc(0, 3, ps_mm, ps_ctx, ps_rs, ppT, prs, ctxT0)
                    ship_ctx(0, ctxT0)

                # ---------- Phase B: attention(b1) + output GEMMs ----------
                with (
                    tc.tile_pool(name="pb_ow", bufs=1) as pow_,
                    tc.tile_pool(name="pb_cf", bufs=2) as pcf,
                    tc.tile_pool(name="pb_o", bufs=2) as po,
                ):
                    ow_sb = pow_.tile([128, N_CC, HID], BF16)
                    OG = 4 * HID
                    for g in range(4):
                        nc.scalar.dma_start(
                            out=ow_sb[:, g * 4 : (g + 1) * 4, :],
                            in_=owt[:, g * OG : (g + 1) * OG],
                        )
                    def load_cf(b):
                        cf = pcf.tile([128, N_CORES, HPC * TOK_SHARD], BF16, tag="cf", name=f"cf{b}")
                        src = cc_out[b].rearrange("j p f -> p j f")
                        nc.sync.dma_start(out=cf[:, 0:4, :], in_=src[:, 0:4, :])
                        nc.sync.dma_start(out=cf[:, 4:8, :], in_=src[:, 4:8, :])
                        return cf

                    cf0 = load_cf(0)

                    def outg_tb(b, cf, tb):
                        o_t = po.tile([128, HID], F32, tag="o", name="o_t")
                        for half in range(2):
                            pso = [
                                ps_mm.tile([128, 512], F32, tag="mm", name=f"pso{nb}")
                                for nb in (2 * half, 2 * half + 1)
                            ]
                            for cc in range(N_CC):
                                j, hh = cc // HPC, cc % HPC
                                c0 = hh * TOK_SHARD + tb * 128
                                for i, nb in enumerate((2 * half, 2 * half + 1)):
                                    nc.tensor.matmul(
                                        pso[i][:],
                                        cf[:, j, c0 : c0 + 128],
                                        ow_sb[:, cc, nb * 512 : (nb + 1) * 512],
                                        start=(cc == 0),
                                        stop=(cc == N_CC - 1),
                                    )
                            for i, nb in enumerate((2 * half, 2 * half + 1)):
                                nc.vector.tensor_add(
                                    o_t[:, nb * 512 : (nb + 1) * 512], pso[i][:],
                                    ob_bc[:, nb * 512 : (nb + 1) * 512],
                                )
                            nc.sync.dma_start(
                                out=out[
                                    b * TOK_SHARD + tb * 128 : b * TOK_SHARD + (tb + 1) * 128,
                                    half * 1024 : (half + 1) * 1024,
                                ],
                                in_=o_t[:, half * 1024 : (half + 1) * 1024],
                            )

                    ctxT1 = pcT.tile([128, HPC, S], BF16, tag="ctxT", name="ctxT1")
                    attn_qc(1, 0, ps_mm, ps_ctx, ps_rs, ppT, prs, ctxT1)
                    attn_qc(1, 1, ps_mm, ps_ctx, ps_rs, ppT, prs, ctxT1)
                    attn_qc(1, 2, ps_mm, ps_ctx, ps_rs, ppT, prs, ctxT1)
                    attn_qc(1, 3, ps_mm, ps_ctx, ps_rs, ppT, prs, ctxT1)
                    ship_ctx(1, ctxT1)
                    # outG(b0) fills the PE while A2A(b1) is in flight
                    outg_tb(0, cf0, 0)
                    cf1 = load_cf(1)
                    outg_tb(0, cf0, 1)
                    outg_tb(1, cf1, 0)
                    outg_tb(1, cf1, 1)

    nc.compile()
    return nc


_CACHE = {}


def _get_nc(apply_mask: bool):
    if apply_mask not in _CACHE:
        _CACHE[apply_mask] = _build(apply_mask)
    return _CACHE[apply_mask]


def _prep_in_maps(input, input_mask, norm_w, norm_b, attn_qkvw, attn_qkvb, attn_ow):
    bf16 = ml_dtypes.bfloat16
    x = np.asarray(input, dtype=np.float32).reshape(T, HID)
    w = np.asarray(attn_qkvw, dtype=np.float32)
    nw = np.asarray(norm_w, dtype=np.float32)
    nb = np.asarray(norm_b, dtype=np.float32)
    qb_ = np.asarray(attn_qkvb, dtype=np.float32)
    ow_f = np.asarray(attn_ow, dtype=np.float32)
    ow = np.ascontiguousarray(ow_f.astype(bf16))
    mask = np.asarray(input_mask, dtype=np.float32).reshape(B, S)

    xt = np.ascontiguousarray(x.T.astype(bf16))  # [HID, T]
    xn = np.ascontiguousarray(x.astype(bf16))  # [T, HID]

    w_eff = nw[:, None] * w  # fold LN gamma into QKV weight
    b_eff = nb @ w + qb_  # fold LN beta into QKV bias
    colsum = w_eff.sum(axis=0)  # [3*HID]
    obias_full = np.ascontiguousarray(
        (b_eff[2 * HID :] @ ow_f).reshape(1, HID).astype(bf16)
    )

    apply_mask = bool(np.any(mask != 0.0))
    if apply_mask:
        # per-key layout: [128 partitions (k within block), B * 16 key-blocks]
        mprep = np.ascontiguousarray(
            mask.reshape(B, S // 128, 128).transpose(2, 0, 1).reshape(128, B * (S // 128))
        )
    in_maps = []
    for i in range(N_CORES):
        cols = []
        for part in range(3):  # q, k, v column shards for this core's heads
            c0 = part * HID + i * HPC * HD
            cols.append(w_eff[:, c0 : c0 + HPC * HD])
        wqkv_i = np.ascontiguousarray(np.concatenate(cols, axis=1).astype(bf16))

        q0 = i * HPC * HD
        k0 = HID + i * HPC * HD
        v0 = 2 * HID + i * HPC * HD
        rqk_i = np.ascontiguousarray(
            np.stack(
                [
                    colsum[q0 : q0 + HD],
                    colsum[q0 + HD : q0 + 2 * HD],
                    -colsum[k0 : k0 + HD],
                    -colsum[k0 + HD : k0 + 2 * HD],
                ],
                axis=1,
            ).astype(np.float32)
        )
        # negated: the ScalarE Identity epilogue computes c1*rq + (-bq)
        bq_i = np.ascontiguousarray(
            np.stack(
                [-b_eff[q0 : q0 + HD], -b_eff[q0 + HD : q0 + 2 * HD]], axis=1
            ).astype(np.float32)
        )
        rv_i = np.ascontiguousarray(
            (-colsum[v0 : v0 + HPC * HD]).reshape(1, HPC * HD).astype(bf16)
        )
        m = {
            "xt": xt,
            "xn": xn,
            "qkvw": wqkv_i,
            "rqk": rqk_i,
            "bq": bq_i,
            "rv": rv_i,
            "obias": obias_full,
            "ow": ow,
        }
        if apply_mask:
            m["imask"] = mprep
        in_maps.append(m)
    return in_maps, apply_mask


def _run(inputs: dict, trace: bool = False):
    from concourse.bass_utils import run_bass_kernel_spmd

    in_maps, apply_mask = _prep_in_maps(**inputs)
    nc = _get_nc(apply_mask)
    res = run_bass_kernel_spmd(nc, in_maps, list(range(N_CORES)), trace=trace)
    out = np.empty((B, S, HID), dtype=np.float32)
    for j in range(N_CORES):
        o = res.results[j]["out"]
        for b in range(B):
            out[b, j * TOK_SHARD : (j + 1) * TOK_SHARD] = o[b * TOK_SHARD : (b + 1) * TOK_SHARD]
    return out, res


def kernel(**inputs) -> np.ndarray:
    out, _ = _run(inputs, trace=False)
    return out
